# revision 36
# baseline (speedup 1.0000x reference)
"""Fused Trainium2 kernel for nn_MultiHeadRelationalModule.

Data-parallel over 8 NeuronCores (8 samples each). The whole per-sample
pipeline (conv1 -> conv2 -> +coords -> K/Q/V proj -> LayerNorm ->
relational attention (4 heads, 596x596) -> softmax -> weighted sum ->
lin1 -> LN -> maxpool -> lin2 -> elu) runs on-chip; the big attention
maps never touch HBM.

v3 engine-balance rework (vs v2 baseline):
  * Act engine runs ONLY Exp/Ln/Relu-family functions (all in the
    natural_log_exp_and_others table) -> a single act-table load for the
    whole kernel (was 19 loads = 24us).
  * LN rsqrt computed as exp(-0.5*ln(var+eps)) on Act (no Sqrt table).
  * at-stage exp is bias-free: exp(z+b) = exp(z)*exp(b); exp(b) folds
    into the following min-op scalar (DVE 4x bf16 mode: 0.26ns/elem).
    Bias-free exp reads PAIRED 2-bank PSUM tiles (half the Act ops).
  * softmax-exp bias exp(alin_b - colsum(alin_w)) folds multiplicatively
    into the V tiles (and their denominator ones-columns), so the second
    exp is also bias-free.
  * Q/K/V projection biases ride a 35th ones-row of feats through the
    projection matmuls; LN means fold into the post-matmul adds; LN
    scales fold into a per-sample copy of qlin/klin rows (rsQ/rsK) and
    into the softmax-normalize multiply (rsV).
  * elu combine ops split across DVE and Pool (Pool reads PSUM fine);
    conv relus + V builds + misc moved to Pool; tail LN stats via
    tensor_scalar accum / tensor_tensor_reduce on DVE.

Key identities:
  elu(x)+1 == max(x + 1, min(exp(x), 1))          (exact)
  A' = elu(z)+1 fed to matmul with alin_w: subtract colsum(alin_w) in
       the softmax bias to undo the +1; that bias is then moved out of
       the exp into a multiplicative row-scale on V.
  max-pool commutes with the final LN (monotone affine map).
"""

import numpy as np
from contextlib import ExitStack

import concourse.bacc as bacc
import concourse.bass as bass
import concourse.mybir as mybir
import concourse.tile as tile
from concourse.bass_utils import run_bass_kernel_spmd

F32 = mybir.dt.float32
BF16 = mybir.dt.bfloat16
FP8 = mybir.dt.float8e4
I32 = mybir.dt.int32
RSQ_MAGIC = 0x5F3759DF
ALSC = 16.0  # alin pre-scale into fp8e4m3 normal range; undone in exp scale
AF = mybir.ActivationFunctionType
ALU = mybir.AluOpType

N_CORES = 8
SPB = 8               # samples per core
N_PIX = 596
HEADS = 4
D = 64
CH = [(0, 128), (128, 256), (256, 384), (384, 512), (512, 596)]
FH = [(0, 512), (512, 596)]
SHIFTS = [(0, 0), (0, 1), (1, 0), (1, 1)]
LN_N = float(HEADS * N_PIX * D)       # 152576
LN2_N = float(N_PIX * D)              # 38144
EPS = 1e-5
W84 = 84 * HEADS

_cache = {}


def _prep_consts(inp):
    """Host-side preprocessing of weights into kernel-friendly layouts."""
    f = np.float32
    c = {}
    conv1_w = np.asarray(inp["conv1_w"], f)
    c["w1s"] = np.ascontiguousarray(
        np.concatenate([conv1_w[:, :, di, dj].T for (di, dj) in SHIFTS], axis=1)
    )  # (4, 64)
    c["b1"] = np.ascontiguousarray(np.asarray(inp["conv1_b"], f)[:, None])  # (16,1)
    conv2_w = np.asarray(inp["conv2_w"], f)
    c["w2s"] = np.ascontiguousarray(
        np.concatenate([conv2_w[:, :, di, dj].T for (di, dj) in SHIFTS], axis=1)
    )  # (16, 128)
    c["b2"] = np.ascontiguousarray(np.asarray(inp["conv2_b"], f)[:, None])  # (32,1)

    p = np.arange(N_PIX)
    c["coords3"] = np.ascontiguousarray(
        np.stack([(p % 4) / 4.0, (p // 4) / 149.0,
                  np.ones(N_PIX)]).astype(f)
    )  # (3, 596): xc, yc, ones-row (projection bias carrier)

    # Q/K projection merged per head with bias in a 35th feats-ones row:
    # cols h*128:h*128+64 = Q, cols h*128+64:h*128+128 = K.
    qp_w = np.asarray(inp["qp_w"], f)
    kp_w = np.asarray(inp["kp_w"], f)
    qp_b = np.asarray(inp["qp_b"], f)
    kp_b = np.asarray(inp["kp_b"], f)
    kqw2e = np.zeros((35, 512), f)
    for h in range(HEADS):
        kqw2e[0:34, h * 128:h * 128 + 64] = qp_w[:, h * 64:(h + 1) * 64]
        kqw2e[0:34, h * 128 + 64:h * 128 + 128] = kp_w[:, h * 64:(h + 1) * 64]
        kqw2e[34, h * 128:h * 128 + 64] = qp_b[h * 64:(h + 1) * 64]
        kqw2e[34, h * 128 + 64:h * 128 + 128] = kp_b[h * 64:(h + 1) * 64]
    c["kqw2e"] = kqw2e

    vwe = np.zeros((35, 256), f)
    vwe[0:34] = np.asarray(inp["vp_w"], f)
    vwe[34] = np.asarray(inp["vp_b"], f)
    c["vwe"] = vwe

    c["qklin"] = np.ascontiguousarray(
        np.concatenate([np.asarray(inp["qlin_w"], f),
                        np.asarray(inp["klin_w"], f)], axis=0)
    )  # (128, 596): rows 0:64 qlin (Q), 64:128 klin (K)

    qkl_b = np.asarray(inp["qlin_b"], f) + np.asarray(inp["klin_b"], f)
    b1tab = np.zeros((128, 5), f)
    ebtab = np.zeros((128, 5), f)
    for ci, (c0, c1) in enumerate(CH):
        b1tab[0:c1 - c0, ci] = qkl_b[c0:c1] + 1.0
        ebtab[0:c1 - c0, ci] = np.exp(qkl_b[c0:c1].astype(np.float64)).astype(f)
    c["b1tab"] = b1tab
    c["ebtab"] = ebtab

    # fp8e4m3 DoubleRowSwInterleave weight pairs for alin rows 0:512 (x16 so
    # the ~0.05-scale entries sit in e4m3's normal range; undone in exp scale).
    import ml_dtypes
    alin_w = np.asarray(inp["alin_w"], f)
    alin16 = np.pad(alin_w * ALSC, ((0, 0), (0, 44)))
    for j in range(2):
        A = alin16[256 * j:256 * j + 128]
        B = alin16[256 * j + 128:256 * j + 256]
        buf = np.zeros((128, 1280), f)
        for ci in range(5):
            c0 = 128 * ci
            blk = np.empty((128, 256), f)
            blk[:, 0::2] = A[:, c0:c0 + 128][:, ::-1]
            blk[:, 1::2] = B[:, c0:c0 + 128][:, ::-1]
            buf[:, 2 * c0:2 * c0 + 256] = blk
        c[f"alin_i8_{j}"] = np.ascontiguousarray(
            buf.astype(ml_dtypes.float8_e4m3))
    c["alin4"] = np.ascontiguousarray(alin_w[512:596, :])  # (84, 596)

    # softmax bias exp(alin_b - colsum(alin_w)) folded into V rows (c2 dim)
    s = np.exp((np.asarray(inp["alin_b"], np.float64)
                - np.asarray(inp["alin_w"], np.float64).sum(axis=0)))
    s = s.astype(f)
    sB = np.zeros((128, 5, 256), f)
    s_cols = np.zeros((128, 5), f)
    for ci, (c0, c1) in enumerate(CH):
        sB[0:c1 - c0, ci, :] = s[c0:c1, None]
        s_cols[0:c1 - c0, ci] = s[c0:c1]
    c["sB"] = sB.reshape(128, 5 * 256)
    c["s_cols"] = s_cols

    l1 = np.zeros((128, 128), f)
    lin1_w = np.asarray(inp["lin1_w"], f)
    l1[:, 0:64] = lin1_w[0:128]
    l1[:, 64:128] = lin1_w[128:256]
    c["lin1w"] = l1
    c["bl1"] = np.ascontiguousarray(np.asarray(inp["lin1_b"], f)[:, None])  # (64,1)
    c["lin2w"] = np.ascontiguousarray(np.asarray(inp["lin2_w"], f))  # (64,10)
    bl2 = np.zeros((10, 2), f)
    bl2[:, 0] = np.asarray(inp["lin2_b"], f)
    bl2[:, 1] = np.asarray(inp["lin2_b"], f) + 1.0
    c["bl2"] = bl2
    c["ones_r"] = np.ones((1, 128), f)
    c["ones_c"] = np.ones((128, 1), f)
    c["epsc"] = np.full((1, 1), EPS, f)
    c["id35"] = np.eye(35, dtype=f)
    # row-group selectors: cols of mm(sel, t62) pick (-mu, rs) per partition
    selqk3 = np.zeros((3, 128), f)
    selqk3[0, 0:64] = 1.0
    selqk3[1, 64:128] = 1.0
    selv3 = np.zeros((3, 128), f)
    selv3[2, :] = 1.0
    c["selqk3"] = selqk3
    c["selv3"] = selv3
    # LN-stat helpers: per tensor T with extended weights W' (35, .):
    # sum(T) = s'^T W' 1 and ssq(T) = <G, W' W'^T> with G = feats' feats'^T
    # (biases included via the ones-row of feats').
    wq = np.concatenate([qp_w, qp_b[None]], 0)
    wk = np.concatenate([kp_w, kp_b[None]], 0)
    wsum3 = np.zeros((35, 3), f)
    wsum3[:, 0] = wq.sum(axis=1)
    wsum3[:, 1] = wk.sum(axis=1)
    wsum3[:, 2] = vwe.sum(axis=1)
    c["wsum3"] = wsum3
    c["gmq"] = np.ascontiguousarray((wq @ wq.T).astype(f))   # (35, 35)
    c["gmk"] = np.ascontiguousarray((wk @ wk.T).astype(f))
    c["gmv"] = np.ascontiguousarray((vwe @ vwe.T).astype(f))
    return c


CONST_SHAPES = {
    "w1s": (4, 64), "b1": (16, 1), "w2s": (16, 128), "b2": (32, 1),
    "coords3": (3, N_PIX), "kqw2e": (35, 512), "vwe": (35, 256),
    "qklin": (128, N_PIX), "b1tab": (128, 5), "ebtab": (128, 5),
    "alin4": (84, N_PIX), "sB": (128, 5 * 256), "s_cols": (128, 5),
    "lin1w": (128, 128), "bl1": (64, 1), "lin2w": (64, 10),
    "bl2": (10, 2), "ones_r": (1, 128), "ones_c": (128, 1), "epsc": (1, 1),
    "id35": (35, 35), "selqk3": (3, 128), "selv3": (3, 128), "wsum3": (35, 3),
    "gmq": (35, 35), "gmk": (35, 35), "gmv": (35, 35),
}
CONST_FP8 = {"alin_i8_0": (128, 1280), "alin_i8_1": (128, 1280)}

# elu min-op engine split per chunk: True -> Pool, False -> DVE (4x)
MIN_POOL = [True, False, True, False, True]


def build_nc(spb=SPB):
    """Build the Bass program (same program runs SPMD on each core)."""
    nc = bacc.Bacc("TRN2", target_bir_lowering=False, debug=False)

    x_dram = nc.dram_tensor("x", [spb, 4, 151, 6], F32, kind="ExternalInput").ap()
    out_dram = nc.dram_tensor("out", [spb, 10], F32, kind="ExternalOutput").ap()
    cdram = {
        k: nc.dram_tensor(k, list(v), F32, kind="ExternalInput").ap()
        for k, v in CONST_SHAPES.items()
    }
    for k, v in CONST_FP8.items():
        cdram[k] = nc.dram_tensor(k, list(v), FP8, kind="ExternalInput").ap()

    with tile.TileContext(nc) as tc, ExitStack() as ctx:
        pc = ctx.enter_context(tc.tile_pool(name="consts", bufs=1))
        # SBUF pools
        px = ctx.enter_context(tc.tile_pool(name="px", bufs=2))
        ph1 = ctx.enter_context(tc.tile_pool(name="ph1", bufs=2))
        pfeat = ctx.enter_context(tc.tile_pool(name="pfeat", bufs=2))
        pstk = ctx.enter_context(tc.tile_pool(name="pstk", bufs=8))
        pqs = ctx.enter_context(tc.tile_pool(name="pqs", bufs=2))
        pbc = ctx.enter_context(tc.tile_pool(name="pbc", bufs=2))
        pv = ctx.enter_context(tc.tile_pool(name="pv", bufs=10))
        pet = ctx.enter_context(tc.tile_pool(name="pet", bufs=3))
        pat = ctx.enter_context(tc.tile_pool(name="pat", bufs=4))
        pext = ctx.enter_context(tc.tile_pool(name="pext", bufs=7))
        psq = ctx.enter_context(tc.tile_pool(name="psq", bufs=2))
        pst = ctx.enter_context(tc.tile_pool(name="pst", bufs=3))
        peall = ctx.enter_context(tc.tile_pool(name="peall", bufs=4))
        pfix = ctx.enter_context(tc.tile_pool(name="pfix", bufs=1))
        # PSUM pools: 8 banks = at-pairs 2x2 + e-ring 2 + eps 1 + fr 1.
        PS = bass.MemorySpace.PSUM
        ps_atp = ctx.enter_context(tc.tile_pool(name="ps_atp", bufs=2, space=PS))
        ps_e = ctx.enter_context(tc.tile_pool(name="ps_e", bufs=2, space=PS))
        ps_eps = ctx.enter_context(tc.tile_pool(name="ps_eps", bufs=1, space=PS))
        ps_fr = ctx.enter_context(tc.tile_pool(name="ps_fr", bufs=1, space=PS))

        # ---- prefetch sample 0's input before the const DMAs ----
        x_t0 = px.tile([4, 151, 6], F32, name="x_t", tag="x")
        nc.sync.dma_start(out=x_t0[:, :, :], in_=x_dram[0])

        # ---- load constants (fp32); critical-path consts first ----
        csb = {}
        first = ["w1s", "b1", "w2s", "b2", "coords3", "kqw2e", "vwe", "qklin",
                 "wsum3", "id35", "selqk3", "selv3", "sB", "s_cols"]
        order = first + [k for k in CONST_SHAPES if k not in first]
        for k in order:
            shp = CONST_SHAPES[k]
            t = pc.tile(list(shp), F32, name=f"c_{k}")
            nc.sync.dma_start(out=t[:, :], in_=cdram[k][:, :])
            csb[k] = t
        alin_i8 = []
        for j in range(2):
            t = pc.tile([128, 1280], FP8, name=f"alin_i8_{j}")
            nc.sync.dma_start(out=t[:, :], in_=cdram[f"alin_i8_{j}"][:, :])
            alin_i8.append(t)

        # ---- one-time bf16 conversions of matmul operands ----
        def to_bf(name, src, shp):
            t = pc.tile(list(shp), BF16, name=name)
            nc.vector.tensor_copy(t[:, :], src[:, :])
            return t

        w1s_bf = to_bf("w1s_bf", csb["w1s"], (4, 64))
        w2s_bf = to_bf("w2s_bf", csb["w2s"], (16, 128))
        kqw2e_bf = to_bf("kqw2e_bf", csb["kqw2e"], (35, 512))
        vwe_bf = to_bf("vwe_bf", csb["vwe"], (35, 256))
        qklin_bf = to_bf("qklin_bf", csb["qklin"], (128, N_PIX))
        lin1w_bf = to_bf("lin1w_bf", csb["lin1w"], (128, 128))
        id35_bf = to_bf("id35_bf", csb["id35"], (35, 35))
        sB_bf = to_bf("sB_bf", csb["sB"], (128, 5 * 256))
        sB3 = sB_bf.rearrange("p (c h d) -> p c (h d)", c=5, h=4)
        alin_bf4 = pc.tile([84, 640], BF16, name="alin_bf4")
        nc.vector.memset(alin_bf4[:, 596:640], 0.0)
        nc.vector.tensor_scalar_mul(alin_bf4[:, 0:N_PIX], csb["alin4"][:, :],
                                    ALSC)
        ones_bf = pc.tile([128, 1], BF16, name="ones_bf")
        nc.vector.memset(ones_bf[:, :], 1.0)
        wsum3_bf = to_bf("wsum3_bf", csb["wsum3"], (35, 3))
        # feats'-transpose staging tiles; col 35 is a persistent ones column
        # so the Gram matmul also yields the feature sums s'.
        ft_bufs = []
        for i in range(3):
            fb = pst.tile([128, 36], BF16, name="ft_sb", tag="ft")
            nc.vector.memset(fb[:, 35:36], 1.0)
            ft_bufs.append(fb)
        emax_all = pfix.tile([64, spb], F32, name="emax_all")

        # feats tiles: conv writes rows 0:32; rows 32:34 coords, row 34 ones,
        # both persistent (written once into each ring buffer).
        feats_bufs = []
        for i in range(2):
            ft = pfeat.tile([35, N_PIX], BF16, name="feats", tag="feats")
            nc.vector.tensor_copy(ft[32:35, :], csb["coords3"][:, :])
            feats_bufs.append(ft)

        # V tiles: cols h*128+64:h*128+128 hold the persistent softmax-scale
        # block s[c2] (denominator ones-columns, pre-scaled).
        for i in range(10):
            vt = pv.tile([128, 512], BF16, name="vt", tag="v")
            vt3 = vt.rearrange("p (h c) -> p h c", c=128)
            ci = i % 5
            csz = CH[ci][1] - CH[ci][0]
            nc.vector.tensor_copy(
                vt3[0:csz, :, 64:128],
                sB3[0:csz, ci, :].rearrange("p (h d) -> p h d", h=4))

        # ================= pipelined per-sample stages =================

        def front_a(s):
            """x load/cast + conv1 + conv2 -> feats (relu on Pool)."""
            S = {"s": s}
            if s == 0:
                x_t = x_t0
            else:
                x_t = px.tile([4, 151, 6], F32, name="x_t", tag="x")
                nc.sync.dma_start(out=x_t[:, :, :], in_=x_dram[s])
            x_bf = px.tile([4, 151, 6], BF16, name="x_bf", tag="xbf")
            nc.gpsimd.tensor_copy(x_bf[:, :, :], x_t[:, :, :])

            h1 = ph1.tile([16, 750], BF16, name="h1", tag="h1")
            h1v = h1.rearrange("c (h w) -> c h w", w=5)
            for (r0, nr, dst0) in ((0, 102, 0), (102, 48, 510)):
                cps = ps_fr.tile([16, nr * 5], F32, name="c1ps", tag="fr")
                for si, (di, dj) in enumerate(SHIFTS):
                    nc.tensor.matmul(
                        cps[:, :],
                        w1s_bf[:, si * 16:(si + 1) * 16],
                        x_bf[:, di + r0:di + r0 + nr, dj:dj + 5],
                        start=(si == 0), stop=(si == 3),
                    )
                nc.vector.tensor_scalar(h1[:, dst0:dst0 + nr * 5], cps[:, :],
                                        csb["b1"][:, 0:1], 0.0,
                                        op0=ALU.add, op1=ALU.max)

            feats = feats_bufs[s % 2]
            for (r0, nr, dst0) in ((0, 128, 0), (128, 21, 512)):
                cps = ps_fr.tile([32, nr * 4], F32, name="c2ps", tag="fr")
                for si, (di, dj) in enumerate(SHIFTS):
                    nc.tensor.matmul(
                        cps[:, :],
                        w2s_bf[:, si * 32:(si + 1) * 32],
                        h1v[:, di + r0:di + r0 + nr, dj:dj + 4],
                        start=(si == 0), stop=(si == 3),
                    )
                nc.vector.tensor_scalar(feats[0:32, dst0:dst0 + nr * 4],
                                        cps[:, :], csb["b2"][:, 0:1], 0.0,
                                        op0=ALU.add, op1=ALU.max)
            S["feats"] = feats
            return S

        def front_b(S):
            """LN stats: G36 = [feats'|1]^T-gram on the PE (last col = s'),
            then ssq = <G, W W^T> via ttr against host Gram mats."""
            feats = S["feats"]
            g_ps = ps_fr.tile([36, 36], F32, name="g_ps", tag="fr")
            for ci, (c0, c1) in enumerate(CH):
                csz = c1 - c0
                ft_ps = ps_e.tile([128, 35], BF16, name="ft_ps", tag="ep")
                nc.tensor.transpose(ft_ps[0:csz, :], feats[:, c0:c1],
                                    id35_bf[:, :])
                ft_sb = ft_bufs[ci % 3]
                nc.vector.tensor_copy(ft_sb[0:csz, 0:35], ft_ps[0:csz, :])
                nc.tensor.matmul(g_ps[:, :], ft_sb[0:csz, :],
                                 ft_sb[0:csz, :],
                                 start=(ci == 0), stop=(ci == 4))
            g_sb = pst.tile([36, 36], BF16, name="g_sb", tag="g_sb")
            nc.vector.tensor_copy(g_sb[:, :], g_ps[:, :])
            gw = psq.tile([35, 3, 35], F32, name="gw", tag="gw")
            acc3 = pst.tile([35, 3], F32, name="acc3", tag="acc3")
            for i, gm in enumerate(("gmq", "gmk", "gmv")):
                nc.gpsimd.tensor_tensor(gw[:, i, :], g_sb[0:35, 0:35],
                                        csb[gm][:, :], op=ALU.mult)
            nc.vector.tensor_reduce(
                acc3[:, :].rearrange("p (a u) -> p a u", u=1),
                gw[:, :, :], axis=mybir.AxisListType.X, op=ALU.add)
            stats_ps = ps_fr.tile([1, 6], F32, name="stats_ps", tag="fr")
            nc.tensor.matmul(stats_ps[0:1, 0:3], g_sb[0:35, 35:36],
                             wsum3_bf[:, :], start=True, stop=True)
            nc.tensor.matmul(stats_ps[0:1, 3:6], csb["ones_c"][0:35, 0:1],
                             acc3[:, :], start=True, stop=True)
            mu3 = pst.tile([1, 3], F32, name="mu3", tag="mu3")
            nc.vector.tensor_scalar_mul(mu3[:, :], stats_ps[0:1, 0:3],
                                        1.0 / LN_N)
            msq3 = pst.tile([1, 3], F32, name="msq3", tag="msq3")
            nc.vector.tensor_scalar_mul(msq3[:, :], stats_ps[0:1, 3:6],
                                        1.0 / LN_N)
            S["mu3"] = mu3
            S["msq3"] = msq3
            return S

        def front_c(S):
            """LN scalars via Newton rsqrt on DVE, then projections."""
            feats, mu3, msq3 = S["feats"], S["mu3"], S["msq3"]
            nmu2 = pst.tile([1, 3], F32, name="nmu2", tag="nmu2")
            nc.vector.scalar_tensor_tensor(nmu2[:, :], mu3[:, :], -1.0,
                                           mu3[:, :],
                                           op0=ALU.mult, op1=ALU.mult)
            var3e = pst.tile([1, 3], F32, name="var3e", tag="var3e")
            nc.vector.scalar_tensor_tensor(var3e[:, :], msq3[:, :], EPS,
                                           nmu2[:, :], op0=ALU.add,
                                           op1=ALU.add)
            nm3 = pst.tile([1, 3], F32, name="nm3", tag="nm3")
            nc.vector.tensor_scalar_mul(nm3[:, :], mu3[:, :], -1.0)
            # transpose (-mu | var) onto partitions 0:3
            t6_ps = ps_fr.tile([3, 2], F32, name="t6_ps", tag="fr")
            nc.tensor.transpose(t6_ps[:, 0:1], nm3[0:1, :],
                                csb["id35"][0:1, 0:1])
            nc.tensor.transpose(t6_ps[:, 1:2], var3e[0:1, :],
                                csb["id35"][0:1, 0:1])
            t62 = pst.tile([3, 2], F32, name="t62", tag="t62")
            nc.vector.tensor_copy(t62[:, :], t6_ps[:, :])
            # fast inverse sqrt + 2 Newton steps: rs = (var+eps)^-0.5
            yk = pst.tile([3, 1], I32, name="yk", tag="yk")
            nc.vector.tensor_scalar(yk[:, :], t62.bitcast(I32)[:, 1:2],
                                    1, None, op0=ALU.logical_shift_right)
            nc.vector.tensor_scalar(yk[:, :], yk[:, :], -1, RSQ_MAGIC,
                                    op0=ALU.mult, op1=ALU.add)
            y = yk.bitcast(F32)
            nt = pst.tile([3, 1], F32, name="nt", tag="nt")
            for _ in range(2):
                nc.vector.tensor_tensor(nt[:, :], y[:, :], y[:, :],
                                        op=ALU.mult)
                nc.vector.tensor_tensor(nt[:, :], nt[:, :], t62[:, 1:2],
                                        op=ALU.mult)
                nc.vector.tensor_scalar(nt[:, :], nt[:, :], -0.5, 1.5,
                                        op0=ALU.mult, op1=ALU.add)
                nc.vector.tensor_tensor(y[:, :], y[:, :], nt[:, :],
                                        op=ALU.mult)
            nc.vector.tensor_copy(t62[:, 1:2], y[:, :])
            # broadcast (-mu, rs) to per-partition columns via row selectors
            bca_ps = ps_fr.tile([128, 4], F32, name="bca_ps", tag="fr")
            nc.tensor.matmul(bca_ps[:, 0:2], csb["selqk3"][:, :], t62[:, :],
                             start=True, stop=True)
            nc.tensor.matmul(bca_ps[:, 2:4], csb["selv3"][:, :], t62[:, :],
                             start=True, stop=True)
            bca = pbc.tile([128, 4], F32, name="bca", tag="bca")
            nc.vector.tensor_copy(bca[:, :], bca_ps[:, :])
            bcqk = bca[:, 0:2]
            bcv = bca[:, 2:4]
            S["bcv"] = bcv

            # per-sample row-scaled qlin/klin (rsQ rows 0:64, rsK rows 64:128)
            qklin_s = pqs.tile([128, N_PIX], BF16, name="qklin_s", tag="qs")
            nc.vector.tensor_scalar_mul(qklin_s[:, :], qklin_bf[:, :],
                                        bcqk[:, 1:2])
            S["qklin_s"] = qklin_s

            # projections; -mu folded into the PSUM->SBUF add
            stacked = []
            for h in range(HEADS):
                st_t = pstk.tile([128, N_PIX], BF16, name="st_t", tag="qk")
                stacked.append(st_t)
                pps = ps_fr.tile([128, 512], F32, name="pps", tag="fr")
                nc.tensor.matmul(pps[:, :], kqw2e_bf[:, h * 128:(h + 1) * 128],
                                 feats[:, 0:512], start=True, stop=True)
                pps2 = ps_e.tile([128, 84], F32, name="pps2", tag="ep")
                nc.tensor.matmul(pps2[:, :], kqw2e_bf[:, h * 128:(h + 1) * 128],
                                 feats[:, 512:596], start=True, stop=True)
                nc.vector.tensor_scalar_add(st_t[:, 0:512], pps[:, :],
                                            bcqk[:, 0:1])
                nc.vector.tensor_scalar_add(st_t[:, 512:596], pps2[:, :],
                                            bcqk[:, 0:1])

            # V = (vps - muV) * s[c2]: Act Identity with per-partition
            # scale s and bias -muV*s (prepped once per sample).
            msv = pst.tile([128, 5], F32, name="msv", tag="msv")
            nc.vector.tensor_scalar(msv[:, :], csb["s_cols"][:, :],
                                    bcv[:, 0:1], None, op0=ALU.mult)
            vtiles = []
            for ci, (c0, c1) in enumerate(CH):
                csz = c1 - c0
                vps = ps_fr.tile([128, 256], F32, name="vps", tag="fr")
                nc.tensor.matmul(vps[0:csz, :], feats[:, c0:c1],
                                 vwe_bf[:, :], start=True, stop=True)
                vt = pv.tile([128, 512], BF16, name="vt", tag="v")
                vt3 = vt.rearrange("p (h c) -> p h c", c=128)
                vps3 = vps.rearrange("p (h c) -> p h c", c=64)
                nc.scalar.activation(vt3[0:csz, :, 0:64], vps3[0:csz, :, :],
                                     AF.Identity,
                                     bias=msv[0:csz, ci:ci + 1],
                                     scale=csb["s_cols"][0:csz, ci:ci + 1])
                vtiles.append(vt)
            S["stacked"] = stacked
            S["vtiles"] = vtiles
            S["eall"] = [peall.tile([128, N_PIX], BF16, name=f"eall{i}",
                                    tag="eall") for i in range(2)]
            return S

        # ---- attention stages (pipeline carried across samples) ----
        def at_pair(S, p, pi, dest_pair):
            """Chunks (2*pi, 2*pi+1): matmuls -> one paired exp -> per-half
            min-mult (DVE 4x) + combine (DVE/Pool split)."""
            w = 512 if not p["merged"] else W84
            atp = ps_atp.tile([128, 2, 512], F32, name="atp", tag="atp")
            for j in range(2):
                ci = 2 * pi + j
                c0, c1 = CH[ci]
                if p["merged"]:
                    for h in range(HEADS):
                        nc.tensor.matmul(atp[:, j, h * 84:(h + 1) * 84],
                                         S["qklin_s"][:, c0:c1],
                                         S["stacked"][h][:, 512:596],
                                         start=True, stop=True)
                else:
                    nc.tensor.matmul(atp[:, j, 0:512],
                                     S["qklin_s"][:, c0:c1],
                                     S["stacked"][p["h"]][:, 0:512],
                                     start=True, stop=True)
            et = pet.tile([128, 2, 512], BF16, name="et", tag="et")
            nc.scalar.activation(et[:, :, 0:w], atp[:, :, 0:w], AF.Exp)
            for j in range(2):
                ci = 2 * pi + j
                eng = nc.gpsimd if MIN_POOL[ci] else nc.vector
                eng.tensor_scalar(et[:, j, 0:w], et[:, j, 0:w],
                                  csb["ebtab"][:, ci:ci + 1], 1.0,
                                  op0=ALU.mult, op1=ALU.min)
                nc.vector.scalar_tensor_tensor(
                    dest_pair[:, j, 0:w], atp[:, j, 0:w],
                    csb["b1tab"][:, ci:ci + 1],
                    et[:, j, 0:w], op0=ALU.add, op1=ALU.max)

        def at_c4(S, p, dest):
            """Chunk 4 (84 c-rows, bf16 dest for the alin remainder)."""
            w = 512 if not p["merged"] else W84
            c0, c1 = CH[4]
            atc = ps_atp.tile([128, 512], F32, name="atc", tag="atp")
            if p["merged"]:
                for h in range(HEADS):
                    nc.tensor.matmul(atc[0:84, h * 84:(h + 1) * 84],
                                     S["qklin_s"][:, c0:c1],
                                     S["stacked"][h][:, 512:596],
                                     start=True, stop=True)
            else:
                nc.tensor.matmul(atc[0:84, 0:512], S["qklin_s"][:, c0:c1],
                                 S["stacked"][p["h"]][:, 0:512],
                                 start=True, stop=True)
            et = pet.tile([128, 512], BF16, name="et4", tag="et4")
            nc.scalar.activation(et[0:84, 0:w], atc[0:84, 0:w], AF.Exp)
            eng = nc.gpsimd if MIN_POOL[4] else nc.vector
            eng.tensor_scalar(et[0:84, 0:w], et[0:84, 0:w],
                              csb["ebtab"][0:84, 4:5], 1.0,
                              op0=ALU.mult, op1=ALU.min)
            nc.vector.scalar_tensor_tensor(
                dest[0:84, 0:w], atc[0:84, 0:w], csb["b1tab"][0:84, 4:5],
                et[0:84, 0:w], op0=ALU.add, op1=ALU.max)

        def e_c2(st, c2i):
            S, p, tiles = st["S"], st["p"], st["tiles"]
            c20, c21 = CH[c2i]
            c2sz = c21 - c20
            w = 512 if not p["merged"] else W84
            if c2i == 0:
                st["eps"] = ps_eps.tile([128, 512], F32, name="eps_t", tag="e")
            eps_t = st["eps"]
            a2ps = ps_e.tile([128, 512], F32, name="a2ps", tag="ep")
            for j in range(2):
                nc.tensor.matmul(
                    a2ps[0:128, 0:w],
                    alin_i8[j][:, 256 * c2i:256 * c2i + 256],
                    tiles[j][:, :, 0:w],
                    start=(j == 0), stop=False,
                    perf_mode=mybir.MatmulPerfMode.DoubleRowSwInterleave)
            nc.tensor.matmul(a2ps[0:128, 0:w],
                             alin_bf4[:, 128 * c2i:128 * c2i + 128],
                             tiles[2][0:84, 0:w],
                             start=False, stop=True)
            ext = pext.tile([128, 512], BF16, name="ext", tag="ext")
            nc.scalar.activation(ext[0:c2sz, 0:w], a2ps[0:c2sz, 0:w], AF.Exp,
                                 scale=1.0 / ALSC)
            if p["merged"]:
                # PSUM accumulation groups must not interleave within a
                # bank's zero region: buffer ext tiles, accumulate in e_tail.
                st.setdefault("exts", []).append(ext)
            else:
                nc.tensor.matmul(eps_t[:, 0:512],
                                 S["vtiles"][c2i][0:c2sz,
                                                  p["h"] * 128:
                                                  (p["h"] + 1) * 128],
                                 ext[0:c2sz, 0:512],
                                 start=(c2i == 0), stop=(c2i == 4))

        def e_tail(st):
            S, p, eps_t = st["S"], st["p"], st["eps"]
            w = 512 if not p["merged"] else W84
            eall = S["eall"]
            rsv = S["bcv"][0:64, 1:2]
            if p["merged"]:
                for h in range(HEADS):
                    for c2i, (c20, c21) in enumerate(CH):
                        c2sz = c21 - c20
                        nc.tensor.matmul(
                            eps_t[:, h * 84:(h + 1) * 84],
                            S["vtiles"][c2i][0:c2sz, h * 128:(h + 1) * 128],
                            st["exts"][c2i][0:c2sz, h * 84:(h + 1) * 84],
                            start=(c2i == 0), stop=(c2i == 4))
            recip64 = pst.tile([64, 512], F32, name="recip64", tag="recip")
            nc.vector.reciprocal(recip64[:, 0:w], eps_t[64:128, 0:w])
            if p["merged"]:
                for h in range(HEADS):
                    nc.vector.scalar_tensor_tensor(
                        eall[h // 2][(h % 2) * 64:(h % 2) * 64 + 64, 512:596],
                        eps_t[0:64, h * 84:(h + 1) * 84], rsv,
                        recip64[:, h * 84:(h + 1) * 84],
                        op0=ALU.mult, op1=ALU.mult)
            else:
                h = p["h"]
                nc.vector.scalar_tensor_tensor(
                    eall[h // 2][(h % 2) * 64:(h % 2) * 64 + 64, 0:512],
                    eps_t[0:64, 0:512], rsv, recip64[:, 0:512],
                    op0=ALU.mult, op1=ALU.mult)

        pending = [None]

        def do_pass(S, p):
            pair0 = pat.tile([128, 2, 512], FP8, name="atp0", tag="atile")
            pair1 = pat.tile([128, 2, 512], FP8, name="atp1", tag="atile")
            at4 = pat.tile([128, 512], BF16, name="at4", tag="a4", bufs=2)
            tiles = [pair0, pair1, at4]
            prev = pending[0]
            at_pair(S, p, 0, pair0)
            if prev is None:
                at_pair(S, p, 1, pair1)
                at_c4(S, p, at4)
            else:
                e_c2(prev, 0)
                e_c2(prev, 1)
                at_pair(S, p, 1, pair1)
                e_c2(prev, 2)
                e_c2(prev, 3)
                at_c4(S, p, at4)
                e_c2(prev, 4)
                e_tail(prev)
            pending[0] = {"S": S, "p": p, "tiles": tiles}

        def flush_pipe():
            prev = pending[0]
            for c2i in range(5):
                e_c2(prev, c2i)
            e_tail(prev)
            pending[0] = None

        def tail_a(S):
            """lin1 + relu (+sum) + sq-sum + max over f-cols 0:512 (ready
            after the four head passes; the merged pass fills 512:596)."""
            eall = S["eall"]
            e2 = psq.tile([64, N_PIX], F32, name="e2", tag="e2")
            lpart = pst.tile([64, 2], F32, name="lpart", tag="lpart")
            acc1 = pst.tile([64, 2], F32, name="acc1", tag="acc1")
            lps = ps_fr.tile([64, 512], F32, name="lps", tag="fr")
            for ck in range(2):
                nc.tensor.matmul(lps[:, :],
                                 lin1w_bf[:, ck * 64:(ck + 1) * 64],
                                 eall[ck][:, 0:512],
                                 start=(ck == 0), stop=(ck == 1))
            nc.scalar.activation(e2[:, 0:512], lps[:, :], AF.Relu,
                                 bias=csb["bl1"][:, 0:1],
                                 accum_out=lpart[:, 0:1])
            sqe = psq.tile([64, N_PIX], F32, name="sqe", tag="sqe")
            nc.gpsimd.tensor_tensor(sqe[:, 0:512], e2[:, 0:512],
                                    e2[:, 0:512], op=ALU.mult)
            nc.vector.tensor_reduce(acc1[:, 1:2], sqe[:, 0:512],
                                    axis=mybir.AxisListType.X, op=ALU.add)
            S["e2"], S["sqe"] = e2, sqe
            S["lpart"], S["acc1"] = lpart, acc1

        def tail_b(S):
            """Remaining f-cols 512:596, then per-sample LN2 scalars and
            the normalized max-pool column."""
            s, eall = S["s"], S["eall"]
            e2, sqe = S["e2"], S["sqe"]
            lpart, acc1 = S["lpart"], S["acc1"]
            lps = ps_fr.tile([64, 512], F32, name="lps", tag="fr")
            for ck in range(2):
                nc.tensor.matmul(lps[:, 0:84],
                                 lin1w_bf[:, ck * 64:(ck + 1) * 64],
                                 eall[ck][:, 512:596],
                                 start=(ck == 0), stop=(ck == 1))
            nc.scalar.activation(e2[:, 512:596], lps[:, 0:84], AF.Relu,
                                 bias=csb["bl1"][:, 0:1],
                                 accum_out=lpart[:, 1:2])
            ls2 = pst.tile([64, 2], F32, name="ls2", tag="ls2")
            nc.vector.tensor_reduce(ls2[:, 0:1], lpart[:, :],
                                    axis=mybir.AxisListType.X, op=ALU.add)
            nc.gpsimd.tensor_tensor(sqe[:, 512:596], e2[:, 512:596],
                                    e2[:, 512:596], op=ALU.mult)
            nc.vector.tensor_reduce(acc1[:, 0:1], sqe[:, 512:596],
                                    axis=mybir.AxisListType.X, op=ALU.add)
            nc.vector.tensor_reduce(ls2[:, 1:2], acc1[:, :],
                                    axis=mybir.AxisListType.X, op=ALU.add)
            emax_s = pst.tile([64, 2], F32, name="emax_s", tag="emax_s")
            nc.vector.tensor_reduce(emax_s[:, 0:1], e2[:, :],
                                    axis=mybir.AxisListType.X, op=ALU.max)
            st2 = ps_fr.tile([1, 2], F32, name="st2", tag="fr")
            nc.tensor.matmul(st2[0:1, :], csb["ones_c"][0:64, 0:1], ls2[:, :],
                             start=True, stop=True)
            # per-sample LN2 scalars (mean/var -> Newton rsqrt)
            m2 = pst.tile([1, 2], F32, name="m2", tag="m2")
            nc.vector.tensor_scalar_mul(m2[:, :], st2[0:1, :], 1.0 / LN2_N)
            ve = pst.tile([1, 2], F32, name="ve", tag="ve")
            nc.vector.scalar_tensor_tensor(ve[:, 1:2], m2[:, 0:1], -1.0,
                                           m2[:, 0:1],
                                           op0=ALU.mult, op1=ALU.mult)
            nc.vector.scalar_tensor_tensor(ve[:, 0:1], m2[:, 1:2], EPS,
                                           ve[:, 1:2], op0=ALU.add,
                                           op1=ALU.add)
            yk2 = pst.tile([1, 1], I32, name="yk2", tag="yk2")
            nc.vector.tensor_scalar(yk2[:, :], ve.bitcast(I32)[:, 0:1],
                                    1, None, op0=ALU.logical_shift_right)
            nc.vector.tensor_scalar(yk2[:, :], yk2[:, :], -1, RSQ_MAGIC,
                                    op0=ALU.mult, op1=ALU.add)
            y2 = yk2.bitcast(F32)
            nt2 = pst.tile([1, 1], F32, name="nt2", tag="nt2")
            for _ in range(2):
                nc.vector.tensor_tensor(nt2[:, :], y2[:, :], y2[:, :],
                                        op=ALU.mult)
                nc.vector.tensor_tensor(nt2[:, :], nt2[:, :], ve[:, 0:1],
                                        op=ALU.mult)
                nc.vector.tensor_scalar(nt2[:, :], nt2[:, :], -0.5, 1.5,
                                        op0=ALU.mult, op1=ALU.add)
                nc.vector.tensor_tensor(y2[:, :], y2[:, :], nt2[:, :],
                                        op=ALU.mult)
            rsnm = pst.tile([1, 2], F32, name="rsnm", tag="rsnm")
            nc.vector.tensor_copy(rsnm[:, 0:1], y2[:, :])
            nc.vector.scalar_tensor_tensor(rsnm[:, 1:2], m2[:, 0:1], -1.0,
                                           y2[:, :], op0=ALU.mult,
                                           op1=ALU.mult)
            bc2_ps = ps_fr.tile([64, 2], F32, name="bc2_ps", tag="fr")
            nc.tensor.matmul(bc2_ps[:, :], csb["ones_r"][0:1, 0:64],
                             rsnm[:, :], start=True, stop=True)
            bc2s = pst.tile([64, 2], F32, name="bc2s", tag="bc2s")
            nc.vector.tensor_copy(bc2s[:, :], bc2_ps[:, :])
            nc.vector.tensor_scalar(emax_all[:, s:s + 1], emax_s[:, 0:1],
                                    bc2s[:, 0:1], bc2s[:, 1:2],
                                    op0=ALU.mult, op1=ALU.add)

        # ---- pipelined schedule: sample s+1's front-end is emitted between
        # sample s's attention passes; the at/e pass pipeline is carried
        # across the sample boundary.
        S = front_a(0)
        front_b(S)
        front_c(S)
        states = {0: S}
        for s in range(spb):
            S = states[s]
            plist = ([dict(h=h, merged=False) for h in range(HEADS)]
                     + [dict(h=None, merged=True)])
            do_pass(S, plist[0])
            if s > 0:
                tail_b(states.pop(s - 1))
            if s + 1 < spb:
                Sn = front_a(s + 1)
            do_pass(S, plist[1])
            if s + 1 < spb:
                front_b(Sn)
            do_pass(S, plist[2])
            if s + 1 < spb:
                front_c(Sn)
                states[s + 1] = Sn
            do_pass(S, plist[3])
            do_pass(S, plist[4])
            tail_a(S)
        flush_pipe()
        tail_b(states.pop(spb - 1))

        # ---------------- lin2 + final elu ----------------
        l2ps = ps_e.tile([10, spb], F32, name="l2ps", tag="ep")
        nc.tensor.matmul(l2ps[:, :], csb["lin2w"][:, :], emax_all[:, :],
                         start=True, stop=True)
        fe = pst.tile([10, spb], F32, name="fe", tag="fe")
        nc.scalar.activation(fe[:, :], l2ps[:, :], AF.Exp,
                             bias=csb["bl2"][:, 0:1])
        nc.vector.tensor_scalar(fe[:, :], fe[:, :], 1.0, -1.0,
                                op0=ALU.min, op1=ALU.add)
        out_sb = pst.tile([10, spb], F32, name="out_sb", tag="out_sb")
        nc.vector.scalar_tensor_tensor(out_sb[:, :], l2ps[:, :],
                                       csb["bl2"][:, 0:1], fe[:, :],
                                       op0=ALU.add, op1=ALU.max)
        nc.sync.dma_start(out=out_dram.rearrange("s t -> t s"), in_=out_sb[:, :])

    return nc


def _reference_numpy(inp):
    """Pure-numpy fallback (only used if LN affine params are nontrivial)."""
    def ln(x, g=None, b=None):
        axes = tuple(range(1, x.ndim))
        mu = x.mean(axis=axes, keepdims=True)
        var = x.var(axis=axes, keepdims=True)
        y = (x - mu) / np.sqrt(var + EPS)
        return y * g + b if g is not None else y

    def elu(x):
        return np.where(x > 0, x, np.expm1(np.minimum(x, 0)))

    x = np.asarray(inp["x"], np.float64)
    N = x.shape[0]
    w1, b1 = np.asarray(inp["conv1_w"], np.float64), np.asarray(inp["conv1_b"], np.float64)
    h = np.zeros((N, 16, 150, 5))
    for di in range(2):
        for dj in range(2):
            h += np.einsum("oc,nchw->nohw", w1[:, :, di, dj],
                           x[:, :, di:di + 150, dj:dj + 5])
    h = np.maximum(h + b1[None, :, None, None], 0)
    w2, b2 = np.asarray(inp["conv2_w"], np.float64), np.asarray(inp["conv2_b"], np.float64)
    h2 = np.zeros((N, 32, 149, 4))
    for di in range(2):
        for dj in range(2):
            h2 += np.einsum("oc,nchw->nohw", w2[:, :, di, dj],
                            h[:, :, di:di + 149, dj:dj + 4])
    h2 = np.maximum(h2 + b2[None, :, None, None], 0)
    p = np.arange(N_PIX)
    xc, yc = (p % 4) / 4.0, (p // 4) / 149.0
    feats = np.concatenate(
        [h2.transpose(0, 2, 3, 1).reshape(N, N_PIX, 32),
         np.broadcast_to(np.stack([xc, yc], 1)[None], (N, N_PIX, 2))], axis=2)

    def proj(wn, bn, gn, bn2):
        P = (feats @ np.asarray(inp[wn], np.float64) + np.asarray(inp[bn], np.float64))
        P = P.reshape(N, N_PIX, HEADS, D).transpose(0, 2, 1, 3)
        return ln(P, np.asarray(inp[gn], np.float64), np.asarray(inp[bn2], np.float64))

    K = proj("kp_w", "kp_b", "knorm_g", "knorm_b")
    Q = proj("qp_w", "qp_b", "qnorm_g", "qnorm_b")
    V = proj("vp_w", "vp_b", "vnorm_g", "vnorm_b")
    A = elu(Q @ np.asarray(inp["qlin_w"], np.float64) + np.asarray(inp["qlin_b"], np.float64)
            + K @ np.asarray(inp["klin_w"], np.float64) + np.asarray(inp["klin_b"], np.float64))
    A = A @ np.asarray(inp["alin_w"], np.float64) + np.asarray(inp["alin_b"], np.float64)
    A = A - A.max(axis=-1, keepdims=True)
    A = np.exp(A)
    A = A / A.sum(axis=-1, keepdims=True)
    E = np.einsum("bhfc,bhcd->bhfd", A, V)
    E = E.transpose(0, 2, 1, 3).reshape(N, N_PIX, HEADS * D)
    E = np.maximum(E @ np.asarray(inp["lin1_w"], np.float64)
                   + np.asarray(inp["lin1_b"], np.float64), 0)
    E = ln(E)
    E = E.max(axis=1)
    out = E @ np.asarray(inp["lin2_w"], np.float64) + np.asarray(inp["lin2_b"], np.float64)
    return elu(out).astype(np.float32)


def kernel(**inputs):
    trivial = (np.all(np.asarray(inputs["knorm_g"]) == 1.0)
               and np.all(np.asarray(inputs["knorm_b"]) == 0.0)
               and np.all(np.asarray(inputs["qnorm_g"]) == 1.0)
               and np.all(np.asarray(inputs["qnorm_b"]) == 0.0)
               and np.all(np.asarray(inputs["vnorm_g"]) == 1.0)
               and np.all(np.asarray(inputs["vnorm_b"]) == 0.0))
    if not trivial:
        return _reference_numpy(inputs)

    x = np.ascontiguousarray(np.asarray(inputs["x"], np.float32))
    n = x.shape[0]
    assert n == N_CORES * SPB, f"expected batch {N_CORES * SPB}, got {n}"
    consts = _prep_consts(inputs)

    if "nc" not in _cache:
        nc = build_nc(SPB)
        nc.compile()
        _cache["nc"] = nc
    nc = _cache["nc"]

    in_maps = []
    for c in range(N_CORES):
        m = dict(consts)
        m["x"] = np.ascontiguousarray(x[c * SPB:(c + 1) * SPB])
        in_maps.append(m)

    import os
    trace = bool(int(os.environ.get("KERNEL_TRACE", "0")))
    res = run_bass_kernel_spmd(nc, in_maps, list(range(N_CORES)), trace=trace)
    kernel._last_results = res
    out = np.concatenate([np.asarray(r["out"]) for r in res.results], axis=0)
    return out.astype(np.float32)


kernel._last_results = None


# revision 50
# speedup vs baseline: 1.0109x; 1.0109x over previous
"""Fused Trainium2 kernel for nn_MultiHeadRelationalModule.

Data-parallel over 8 NeuronCores (8 samples each). The whole per-sample
pipeline (conv1 -> conv2 -> +coords -> K/Q/V proj -> LayerNorm ->
relational attention (4 heads, 596x596) -> softmax -> weighted sum ->
lin1 -> LN -> maxpool -> lin2 -> elu) runs on-chip; the big attention
maps never touch HBM.

v3 engine-balance rework (vs v2 baseline):
  * Act engine runs ONLY Exp/Ln/Relu-family functions (all in the
    natural_log_exp_and_others table) -> a single act-table load for the
    whole kernel (was 19 loads = 24us).
  * LN rsqrt computed as exp(-0.5*ln(var+eps)) on Act (no Sqrt table).
  * at-stage exp is bias-free: exp(z+b) = exp(z)*exp(b); exp(b) folds
    into the following min-op scalar (DVE 4x bf16 mode: 0.26ns/elem).
    Bias-free exp reads PAIRED 2-bank PSUM tiles (half the Act ops).
  * softmax-exp bias exp(alin_b - colsum(alin_w)) folds multiplicatively
    into the V tiles (and their denominator ones-columns), so the second
    exp is also bias-free.
  * Q/K/V projection biases ride a 35th ones-row of feats through the
    projection matmuls; LN means fold into the post-matmul adds; LN
    scales fold into a per-sample copy of qlin/klin rows (rsQ/rsK) and
    into the softmax-normalize multiply (rsV).
  * elu combine ops split across DVE and Pool (Pool reads PSUM fine);
    conv relus + V builds + misc moved to Pool; tail LN stats via
    tensor_scalar accum / tensor_tensor_reduce on DVE.

Key identities:
  elu(x)+1 == max(x + 1, min(exp(x), 1))          (exact)
  A' = elu(z)+1 fed to matmul with alin_w: subtract colsum(alin_w) in
       the softmax bias to undo the +1; that bias is then moved out of
       the exp into a multiplicative row-scale on V.
  max-pool commutes with the final LN (monotone affine map).
"""

import numpy as np
from contextlib import ExitStack

import concourse.bacc as bacc
import concourse.bass as bass
import concourse.mybir as mybir
import concourse.tile as tile
from concourse.bass_utils import run_bass_kernel_spmd

F32 = mybir.dt.float32
BF16 = mybir.dt.bfloat16
FP8 = mybir.dt.float8e4
I32 = mybir.dt.int32
RSQ_MAGIC = 0x5F3759DF
ALSC = 16.0  # alin pre-scale into fp8e4m3 normal range; undone in exp scale
AF = mybir.ActivationFunctionType
ALU = mybir.AluOpType

N_CORES = 8
SPB = 8               # samples per core
N_PIX = 596
HEADS = 4
D = 64
CH = [(0, 128), (128, 256), (256, 384), (384, 512), (512, 596)]
FH = [(0, 512), (512, 596)]
SHIFTS = [(0, 0), (0, 1), (1, 0), (1, 1)]
LN_N = float(HEADS * N_PIX * D)       # 152576
LN2_N = float(N_PIX * D)              # 38144
EPS = 1e-5
W84 = 84 * HEADS

_cache = {}


def _prep_consts(inp):
    """Host-side preprocessing of weights into kernel-friendly layouts."""
    f = np.float32
    c = {}
    conv1_w = np.asarray(inp["conv1_w"], f)
    c["w1s"] = np.ascontiguousarray(
        np.concatenate([conv1_w[:, :, di, dj].T for (di, dj) in SHIFTS], axis=1)
    )  # (4, 64)
    c["b1"] = np.ascontiguousarray(np.asarray(inp["conv1_b"], f)[:, None])  # (16,1)
    conv2_w = np.asarray(inp["conv2_w"], f)
    c["w2s"] = np.ascontiguousarray(
        np.concatenate([conv2_w[:, :, di, dj].T for (di, dj) in SHIFTS], axis=1)
    )  # (16, 128)
    c["b2"] = np.ascontiguousarray(np.asarray(inp["conv2_b"], f)[:, None])  # (32,1)

    p = np.arange(N_PIX)
    c["coords3"] = np.ascontiguousarray(
        np.stack([(p % 4) / 4.0, (p // 4) / 149.0,
                  np.ones(N_PIX)]).astype(f)
    )  # (3, 596): xc, yc, ones-row (projection bias carrier)

    # Q/K projection merged per head with bias in a 35th feats-ones row:
    # cols h*128:h*128+64 = Q, cols h*128+64:h*128+128 = K.
    qp_w = np.asarray(inp["qp_w"], f)
    kp_w = np.asarray(inp["kp_w"], f)
    qp_b = np.asarray(inp["qp_b"], f)
    kp_b = np.asarray(inp["kp_b"], f)
    kqw2e = np.zeros((35, 512), f)
    for h in range(HEADS):
        kqw2e[0:34, h * 128:h * 128 + 64] = qp_w[:, h * 64:(h + 1) * 64]
        kqw2e[0:34, h * 128 + 64:h * 128 + 128] = kp_w[:, h * 64:(h + 1) * 64]
        kqw2e[34, h * 128:h * 128 + 64] = qp_b[h * 64:(h + 1) * 64]
        kqw2e[34, h * 128 + 64:h * 128 + 128] = kp_b[h * 64:(h + 1) * 64]
    c["kqw2e"] = kqw2e

    vwe = np.zeros((35, 256), f)
    vwe[0:34] = np.asarray(inp["vp_w"], f)
    vwe[34] = np.asarray(inp["vp_b"], f)
    c["vwe"] = vwe

    c["qklin"] = np.ascontiguousarray(
        np.concatenate([np.asarray(inp["qlin_w"], f),
                        np.asarray(inp["klin_w"], f)], axis=0)
    )  # (128, 596): rows 0:64 qlin (Q), 64:128 klin (K)

    qkl_b = np.asarray(inp["qlin_b"], f) + np.asarray(inp["klin_b"], f)
    b1tab = np.zeros((128, 5), f)
    ebtab = np.zeros((128, 5), f)
    for ci, (c0, c1) in enumerate(CH):
        b1tab[0:c1 - c0, ci] = qkl_b[c0:c1] + 1.0
        ebtab[0:c1 - c0, ci] = np.exp(qkl_b[c0:c1].astype(np.float64)).astype(f)
    c["b1tab"] = b1tab
    c["ebtab"] = ebtab

    # fp8e4m3 DoubleRowSwInterleave weight pairs for alin rows 0:512 (x16 so
    # the ~0.05-scale entries sit in e4m3's normal range; undone in exp scale).
    import ml_dtypes
    alin_w = np.asarray(inp["alin_w"], f)
    alin16 = np.pad(alin_w * ALSC, ((0, 0), (0, 44)))
    for j in range(2):
        A = alin16[256 * j:256 * j + 128]
        B = alin16[256 * j + 128:256 * j + 256]
        buf = np.zeros((128, 1280), f)
        for ci in range(5):
            c0 = 128 * ci
            blk = np.empty((128, 256), f)
            blk[:, 0::2] = A[:, c0:c0 + 128][:, ::-1]
            blk[:, 1::2] = B[:, c0:c0 + 128][:, ::-1]
            buf[:, 2 * c0:2 * c0 + 256] = blk
        c[f"alin_i8_{j}"] = np.ascontiguousarray(
            buf.astype(ml_dtypes.float8_e4m3))
    c["alin4"] = np.ascontiguousarray(
        np.pad(alin_w[512:596, :] * ALSC, ((0, 0), (0, 44))
               ).astype(ml_dtypes.bfloat16))  # (84, 640), pre-scaled

    # softmax bias exp(alin_b - colsum(alin_w)) folded into V rows (c2 dim)
    s = np.exp((np.asarray(inp["alin_b"], np.float64)
                - np.asarray(inp["alin_w"], np.float64).sum(axis=0)))
    s = s.astype(f)
    s_cols = np.zeros((128, 5), f)
    for ci, (c0, c1) in enumerate(CH):
        s_cols[0:c1 - c0, ci] = s[c0:c1]
    c["s_cols"] = s_cols

    lin1_w = np.asarray(inp["lin1_w"], f)
    l1 = np.zeros((64, 256), f)
    for h in range(HEADS):
        l1[:, h * 64:(h + 1) * 64] = lin1_w[h * 64:(h + 1) * 64, :]
    c["lin1w"] = l1
    c["bl1"] = np.ascontiguousarray(np.asarray(inp["lin1_b"], f)[:, None])  # (64,1)
    b65 = np.zeros((65, 64), f)
    b65[64, :] = np.asarray(inp["lin1_b"], f)
    c["bl1r65"] = np.ascontiguousarray(b65.astype(ml_dtypes.bfloat16))
    c["lin2w"] = np.ascontiguousarray(np.asarray(inp["lin2_w"], f))  # (64,10)
    bl2 = np.zeros((10, 2), f)
    bl2[:, 0] = np.asarray(inp["lin2_b"], f)
    bl2[:, 1] = np.asarray(inp["lin2_b"], f) + 1.0
    c["bl2"] = bl2
    c["ones_r"] = np.ones((1, 128), f)
    c["ones_c"] = np.ones((128, 1), f)
    c["epsc"] = np.full((1, 1), EPS, f)
    c["id35"] = np.eye(35, dtype=f)
    # row-group selectors: cols of mm(sel, t62) pick (-mu, rs) per partition
    selqk3 = np.zeros((3, 128), f)
    selqk3[0, 0:64] = 1.0
    selqk3[1, 64:128] = 1.0
    selv3 = np.zeros((3, 128), f)
    selv3[2, :] = 1.0
    c["selqk3"] = selqk3
    c["selv3"] = selv3
    # LN-stat helpers: per tensor T with extended weights W' (35, .):
    # sum(T) = s'^T W' 1 and ssq(T) = <G, W' W'^T> with G = feats' feats'^T
    # (biases included via the ones-row of feats').
    wq = np.concatenate([qp_w, qp_b[None]], 0)
    wk = np.concatenate([kp_w, kp_b[None]], 0)
    wsum3 = np.zeros((35, 3), f)
    wsum3[:, 0] = wq.sum(axis=1)
    wsum3[:, 1] = wk.sum(axis=1)
    wsum3[:, 2] = vwe.sum(axis=1)
    c["wsum3"] = wsum3.astype(ml_dtypes.bfloat16)
    for k in ("w1s", "w2s", "coords3", "kqw2e", "vwe", "qklin", "lin1w",
              "id35"):
        c[k] = c[k].astype(ml_dtypes.bfloat16)
    c["gmq"] = np.ascontiguousarray((wq @ wq.T).astype(f))   # (35, 35)
    c["gmk"] = np.ascontiguousarray((wk @ wk.T).astype(f))
    c["gmv"] = np.ascontiguousarray((vwe @ vwe.T).astype(f))
    return c


CONST_SHAPES = {
    "b1": (16, 1), "b2": (32, 1),
    "b1tab": (128, 5), "ebtab": (128, 5), "s_cols": (128, 5),
    "bl1": (64, 1), "lin2w": (64, 10),
    "bl2": (10, 2), "ones_r": (1, 128), "ones_c": (128, 1), "epsc": (1, 1),
    "selqk3": (3, 128), "selv3": (3, 128),
    "gmq": (35, 35), "gmk": (35, 35), "gmv": (35, 35),
}
CONST_BF16 = {
    "bl1r65": (65, 64),
    "w1s": (4, 64), "w2s": (16, 128), "coords3": (3, N_PIX),
    "kqw2e": (35, 512), "vwe": (35, 256), "qklin": (128, N_PIX),
    "lin1w": (64, 256), "alin4": (84, 640), "wsum3": (35, 3),
    "id35": (35, 35),
}
CONST_FP8 = {"alin_i8_0": (128, 1280), "alin_i8_1": (128, 1280)}

# elu min-op engine split per chunk: True -> Pool, False -> DVE (4x)
MIN_POOL = [False, False, False, False, False]


def build_nc(spb=SPB):
    """Build the Bass program (same program runs SPMD on each core)."""
    nc = bacc.Bacc("TRN2", target_bir_lowering=False, debug=False)

    x_dram = nc.dram_tensor("x", [spb, 4, 151, 6], F32, kind="ExternalInput").ap()
    out_dram = nc.dram_tensor("out", [spb, 10], F32, kind="ExternalOutput").ap()
    cdram = {
        k: nc.dram_tensor(k, list(v), F32, kind="ExternalInput").ap()
        for k, v in CONST_SHAPES.items()
    }
    for k, v in CONST_BF16.items():
        cdram[k] = nc.dram_tensor(k, list(v), BF16, kind="ExternalInput").ap()
    for k, v in CONST_FP8.items():
        cdram[k] = nc.dram_tensor(k, list(v), FP8, kind="ExternalInput").ap()

    with tile.TileContext(nc) as tc, ExitStack() as ctx:
        pc = ctx.enter_context(tc.tile_pool(name="consts", bufs=1))
        # SBUF pools
        px = ctx.enter_context(tc.tile_pool(name="px", bufs=2))
        ph1 = ctx.enter_context(tc.tile_pool(name="ph1", bufs=2))
        pfeat = ctx.enter_context(tc.tile_pool(name="pfeat", bufs=2))
        pstk = ctx.enter_context(tc.tile_pool(name="pstk", bufs=8))
        pqs = ctx.enter_context(tc.tile_pool(name="pqs", bufs=2))
        pbc = ctx.enter_context(tc.tile_pool(name="pbc", bufs=2))
        pv = ctx.enter_context(tc.tile_pool(name="pv", bufs=10))
        pet = ctx.enter_context(tc.tile_pool(name="pet", bufs=3))
        pat = ctx.enter_context(tc.tile_pool(name="pat", bufs=4))
        pext = ctx.enter_context(tc.tile_pool(name="pext", bufs=7))
        psq = ctx.enter_context(tc.tile_pool(name="psq", bufs=2))
        pst = ctx.enter_context(tc.tile_pool(name="pst", bufs=3))
        peall = ctx.enter_context(tc.tile_pool(name="peall", bufs=8))
        ptl = ctx.enter_context(tc.tile_pool(name="ptl", bufs=2))
        pfix = ctx.enter_context(tc.tile_pool(name="pfix", bufs=1))
        # PSUM pools: 8 banks = at-pairs 2x2 + e-ring 2 + eps 1 + fr 1.
        PS = bass.MemorySpace.PSUM
        ps_atp = ctx.enter_context(tc.tile_pool(name="ps_atp", bufs=2, space=PS))
        ps_e = ctx.enter_context(tc.tile_pool(name="ps_e", bufs=2, space=PS))
        ps_eps = ctx.enter_context(tc.tile_pool(name="ps_eps", bufs=1, space=PS))
        ps_fr = ctx.enter_context(tc.tile_pool(name="ps_fr", bufs=1, space=PS))

        # ---- prefetch sample 0's input before the const DMAs ----
        x_t0 = px.tile([4, 151, 6], F32, name="x_t", tag="x")
        nc.sync.dma_start(out=x_t0[:, :, :], in_=x_dram[0])

        # ---- load constants; critical-path consts first ----
        csb = {}
        first = ["w1s", "b1", "w2s", "b2", "coords3", "kqw2e", "vwe",
                 "qklin", "wsum3", "id35", "selqk3", "selv3", "s_cols",
                 "b1tab", "ebtab", "gmq", "gmk", "gmv"]
        order = first + [k for k in list(CONST_SHAPES) + list(CONST_BF16)
                         if k not in first]
        for k in order:
            if k in CONST_SHAPES:
                shp, dt = CONST_SHAPES[k], F32
            else:
                shp, dt = CONST_BF16[k], BF16
            t = pc.tile(list(shp), dt, name=f"c_{k}")
            nc.sync.dma_start(out=t[:, :], in_=cdram[k][:, :])
            csb[k] = t
        alin_i8 = []
        for j in range(2):
            t = pc.tile([128, 1280], FP8, name=f"alin_i8_{j}")
            nc.sync.dma_start(out=t[:, :], in_=cdram[f"alin_i8_{j}"][:, :])
            alin_i8.append(t)

        w1s_bf = csb["w1s"]
        w2s_bf = csb["w2s"]
        kqw2e_bf = csb["kqw2e"]
        vwe_bf = csb["vwe"]
        qklin_bf = csb["qklin"]
        lin1w_bf = csb["lin1w"]
        id35_bf = csb["id35"]
        alin_bf4 = csb["alin4"]
        wsum3_bf = csb["wsum3"]
        ones_bf = pc.tile([128, 1], BF16, name="ones_bf")
        nc.vector.memset(ones_bf[:, :], 1.0)
        # sB: per-c2-chunk softmax scale broadcast, built from s_cols
        ones256 = pc.tile([128, 256], BF16, name="ones256")
        nc.vector.memset(ones256[:, :], 1.0)
        sB_bf = pc.tile([128, 5, 256], BF16, name="sB_bf")
        for ci in range(5):
            nc.vector.tensor_scalar_mul(sB_bf[:, ci, :], ones256[:, :],
                                        csb["s_cols"][:, ci:ci + 1])
        sB3 = sB_bf
        # feats'-transpose staging tiles; col 35 is a persistent ones column
        # so the Gram matmul also yields the feature sums s'.
        ft_bufs = []
        for i in range(3):
            fb = pst.tile([128, 36], BF16, name="ft_sb", tag="ft")
            nc.vector.memset(fb[:, 35:36], 1.0)
            ft_bufs.append(fb)
        emax_all = pfix.tile([64, spb], F32, name="emax_all")

        # eall ring: row 64 is a persistent ones-row (bias carrier for the
        # transposed lin1); the per-pass normalize writes only rows 0:64.
        for i in range(8):
            eb_t = peall.tile([65, N_PIX], BF16, name="eall_i", tag="eall")
            nc.vector.memset(eb_t[64:65, :], 1.0)

        # feats tiles: conv writes rows 0:32; rows 32:34 coords, row 34 ones,
        # both persistent (written once into each ring buffer).
        feats_bufs = []
        for i in range(2):
            ft = pfeat.tile([35, N_PIX], BF16, name="feats", tag="feats")
            nc.vector.tensor_copy(ft[32:35, :], csb["coords3"][:, :])
            feats_bufs.append(ft)

        # V tiles: cols h*128+64:h*128+128 hold the persistent softmax-scale
        # block s[c2] (denominator ones-columns, pre-scaled).
        for i in range(10):
            vt = pv.tile([128, 512], BF16, name="vt", tag="v")
            vt3 = vt.rearrange("p (h c) -> p h c", c=128)
            ci = i % 5
            csz = CH[ci][1] - CH[ci][0]
            nc.vector.tensor_copy(
                vt3[0:csz, :, 64:128],
                sB3[0:csz, ci, :].rearrange("p (h d) -> p h d", h=4))

        # ================= pipelined per-sample stages =================

        def front_a(s):
            """x load/cast + conv1 + conv2 -> feats (relu on Pool)."""
            S = {"s": s}
            if s == 0:
                x_t = x_t0
            else:
                x_t = px.tile([4, 151, 6], F32, name="x_t", tag="x")
                nc.sync.dma_start(out=x_t[:, :, :], in_=x_dram[s])
            x_bf = px.tile([4, 151, 6], BF16, name="x_bf", tag="xbf")
            nc.gpsimd.tensor_copy(x_bf[:, :, :], x_t[:, :, :])

            h1 = ph1.tile([16, 750], BF16, name="h1", tag="h1")
            h1v = h1.rearrange("c (h w) -> c h w", w=5)
            for (r0, nr, dst0) in ((0, 102, 0), (102, 48, 510)):
                cps = ps_fr.tile([16, nr * 5], F32, name="c1ps", tag="fr")
                for si, (di, dj) in enumerate(SHIFTS):
                    nc.tensor.matmul(
                        cps[:, :],
                        w1s_bf[:, si * 16:(si + 1) * 16],
                        x_bf[:, di + r0:di + r0 + nr, dj:dj + 5],
                        start=(si == 0), stop=(si == 3),
                    )
                nc.vector.tensor_scalar(h1[:, dst0:dst0 + nr * 5], cps[:, :],
                                        csb["b1"][:, 0:1], 0.0,
                                        op0=ALU.add, op1=ALU.max)

            feats = feats_bufs[s % 2]
            for (r0, nr, dst0) in ((0, 128, 0), (128, 21, 512)):
                cps = ps_fr.tile([32, nr * 4], F32, name="c2ps", tag="fr")
                for si, (di, dj) in enumerate(SHIFTS):
                    nc.tensor.matmul(
                        cps[:, :],
                        w2s_bf[:, si * 32:(si + 1) * 32],
                        h1v[:, di + r0:di + r0 + nr, dj:dj + 4],
                        start=(si == 0), stop=(si == 3),
                    )
                nc.vector.tensor_scalar(feats[0:32, dst0:dst0 + nr * 4],
                                        cps[:, :], csb["b2"][:, 0:1], 0.0,
                                        op0=ALU.add, op1=ALU.max)
            S["feats"] = feats
            return S

        def front_b(S):
            """LN stats: G36 = [feats'|1]^T-gram on the PE (last col = s'),
            then ssq = <G, W W^T> via ttr against host Gram mats."""
            feats = S["feats"]
            g_ps = ps_fr.tile([36, 36], F32, name="g_ps", tag="fr")
            for ci, (c0, c1) in enumerate(CH):
                csz = c1 - c0
                ft_ps = ps_e.tile([128, 35], BF16, name="ft_ps", tag="ep")
                nc.tensor.transpose(ft_ps[0:csz, :], feats[:, c0:c1],
                                    id35_bf[:, :])
                ft_sb = ft_bufs[ci % 3]
                nc.vector.tensor_copy(ft_sb[0:csz, 0:35], ft_ps[0:csz, :])
                nc.tensor.matmul(g_ps[:, :], ft_sb[0:csz, :],
                                 ft_sb[0:csz, :],
                                 start=(ci == 0), stop=(ci == 4))
            g_sb = pst.tile([36, 36], BF16, name="g_sb", tag="g_sb")
            nc.vector.tensor_copy(g_sb[:, :], g_ps[:, :])
            gw = psq.tile([35, 3, 35], F32, name="gw", tag="gw")
            acc3 = pst.tile([35, 3], F32, name="acc3", tag="acc3")
            for i, gm in enumerate(("gmq", "gmk", "gmv")):
                nc.gpsimd.tensor_tensor(gw[:, i, :], g_sb[0:35, 0:35],
                                        csb[gm][:, :], op=ALU.mult)
            nc.vector.tensor_reduce(
                acc3[:, :].rearrange("p (a u) -> p a u", u=1),
                gw[:, :, :], axis=mybir.AxisListType.X, op=ALU.add)
            stats_ps = ps_fr.tile([1, 6], F32, name="stats_ps", tag="fr")
            nc.tensor.matmul(stats_ps[0:1, 0:3], g_sb[0:35, 35:36],
                             wsum3_bf[:, :], start=True, stop=True)
            nc.tensor.matmul(stats_ps[0:1, 3:6], csb["ones_c"][0:35, 0:1],
                             acc3[:, :], start=True, stop=True)
            mu3 = pst.tile([1, 3], F32, name="mu3", tag="mu3")
            nc.vector.tensor_scalar_mul(mu3[:, :], stats_ps[0:1, 0:3],
                                        1.0 / LN_N)
            msq3 = pst.tile([1, 3], F32, name="msq3", tag="msq3")
            nc.vector.tensor_scalar_mul(msq3[:, :], stats_ps[0:1, 3:6],
                                        1.0 / LN_N)
            S["mu3"] = mu3
            S["msq3"] = msq3
            return S

        def front_c1(S):
            """LN scalars via Newton rsqrt on DVE."""
            mu3, msq3 = S["mu3"], S["msq3"]
            nmu2 = pst.tile([1, 3], F32, name="nmu2", tag="nmu2")
            nc.vector.scalar_tensor_tensor(nmu2[:, :], mu3[:, :], -1.0,
                                           mu3[:, :],
                                           op0=ALU.mult, op1=ALU.mult)
            var3e = pst.tile([1, 3], F32, name="var3e", tag="var3e")
            nc.vector.scalar_tensor_tensor(var3e[:, :], msq3[:, :], EPS,
                                           nmu2[:, :], op0=ALU.add,
                                           op1=ALU.add)
            nm3 = pst.tile([1, 3], F32, name="nm3", tag="nm3")
            nc.vector.tensor_scalar_mul(nm3[:, :], mu3[:, :], -1.0)
            # transpose (-mu | var) onto partitions 0:3
            t6_ps = ps_fr.tile([3, 2], F32, name="t6_ps", tag="fr")
            nc.tensor.transpose(t6_ps[:, 0:1], nm3[0:1, :],
                                csb["ones_c"][0:1, 0:1])
            nc.tensor.transpose(t6_ps[:, 1:2], var3e[0:1, :],
                                csb["ones_c"][0:1, 0:1])
            t62 = pst.tile([3, 2], F32, name="t62", tag="t62")
            nc.vector.tensor_copy(t62[:, :], t6_ps[:, :])
            # fast inverse sqrt + 2 Newton steps: rs = (var+eps)^-0.5
            yk = pst.tile([3, 1], I32, name="yk", tag="yk")
            nc.vector.tensor_scalar(yk[:, :], t62.bitcast(I32)[:, 1:2],
                                    1, None, op0=ALU.logical_shift_right)
            nc.vector.tensor_scalar(yk[:, :], yk[:, :], -1, RSQ_MAGIC,
                                    op0=ALU.mult, op1=ALU.add)
            y = yk.bitcast(F32)
            nt = pst.tile([3, 1], F32, name="nt", tag="nt")
            for _ in range(2):
                nc.vector.tensor_tensor(nt[:, :], y[:, :], y[:, :],
                                        op=ALU.mult)
                nc.vector.tensor_tensor(nt[:, :], nt[:, :], t62[:, 1:2],
                                        op=ALU.mult)
                nc.vector.tensor_scalar(nt[:, :], nt[:, :], -0.5, 1.5,
                                        op0=ALU.mult, op1=ALU.add)
                nc.vector.tensor_tensor(y[:, :], y[:, :], nt[:, :],
                                        op=ALU.mult)
            nc.vector.tensor_copy(t62[:, 1:2], y[:, :])
            # broadcast (-mu, rs) to per-partition columns via row selectors
            bca_ps = ps_fr.tile([128, 4], F32, name="bca_ps", tag="fr")
            nc.tensor.matmul(bca_ps[:, 0:2], csb["selqk3"][:, :], t62[:, :],
                             start=True, stop=True)
            nc.tensor.matmul(bca_ps[:, 2:4], csb["selv3"][:, :], t62[:, :],
                             start=True, stop=True)
            bca = pbc.tile([128, 4], F32, name="bca", tag="bca")
            nc.vector.tensor_copy(bca[:, :], bca_ps[:, :])
            S["bca"] = bca
            S["bcv"] = bca[:, 2:4]

            # per-sample row-scaled qlin/klin (rsQ rows 0:64, rsK rows 64:128)
            qklin_s = pqs.tile([128, N_PIX], BF16, name="qklin_s", tag="qs")
            nc.vector.tensor_scalar_mul(qklin_s[:, :], qklin_bf[:, :],
                                        bca[:, 1:2])
            S["qklin_s"] = qklin_s


        def front_c2(S):
            """Projections; -mu folded into the PSUM->SBUF add."""
            feats = S["feats"]
            bca = S["bca"]
            bcqk = bca[:, 0:2]
            bcv = bca[:, 2:4]
            stacked = []
            for h in range(HEADS):
                st_t = pstk.tile([128, N_PIX], BF16, name="st_t", tag="qk")
                stacked.append(st_t)
                pps = ps_fr.tile([128, 512], F32, name="pps", tag="fr")
                nc.tensor.matmul(pps[:, :], kqw2e_bf[:, h * 128:(h + 1) * 128],
                                 feats[:, 0:512], start=True, stop=True)
                pps2 = ps_e.tile([128, 84], F32, name="pps2", tag="ep")
                nc.tensor.matmul(pps2[:, :], kqw2e_bf[:, h * 128:(h + 1) * 128],
                                 feats[:, 512:596], start=True, stop=True)
                nc.vector.tensor_scalar_add(st_t[:, 0:512], pps[:, :],
                                            bcqk[:, 0:1])
                nc.vector.tensor_scalar_add(st_t[:, 512:596], pps2[:, :],
                                            bcqk[:, 0:1])

            # V = (vps - muV) * s[c2]: Act Identity with per-partition
            # scale s and bias -muV*s (prepped once per sample).
            msv = pst.tile([128, 5], F32, name="msv", tag="msv")
            nc.vector.tensor_scalar(msv[:, :], csb["s_cols"][:, :],
                                    bcv[:, 0:1], None, op0=ALU.mult)
            vtiles = []
            for ci, (c0, c1) in enumerate(CH):
                csz = c1 - c0
                vps = ps_fr.tile([128, 256], F32, name="vps", tag="fr")
                nc.tensor.matmul(vps[0:csz, :], feats[:, c0:c1],
                                 vwe_bf[:, :], start=True, stop=True)
                vt = pv.tile([128, 512], BF16, name="vt", tag="v")
                vt3 = vt.rearrange("p (h c) -> p h c", c=128)
                vps3 = vps.rearrange("p (h c) -> p h c", c=64)
                nc.scalar.activation(vt3[0:csz, :, 0:64], vps3[0:csz, :, :],
                                     AF.Identity,
                                     bias=msv[0:csz, ci:ci + 1],
                                     scale=csb["s_cols"][0:csz, ci:ci + 1])
                vtiles.append(vt)
            S["stacked"] = stacked
            S["vtiles"] = vtiles
            S["eall"] = [peall.tile([65, N_PIX], BF16, name=f"eall{i}",
                                    tag="eall") for i in range(HEADS)]
            stats = ptl.tile([128, 10], F32, name="stats128", tag="stats")
            nc.vector.memset(stats[64:128, :], 0.0)
            S["stats128"] = stats
            S["pm"] = ptl.tile([1, 5, 64], F32, name="pm", tag="pm")
            return S

        # ---- attention stages (pipeline carried across samples) ----
        def at_pair_mm(S, p, pi):
            atp = ps_atp.tile([128, 2, 512], F32, name="atp", tag="atp")
            for j in range(2):
                ci = 2 * pi + j
                c0, c1 = CH[ci]
                if p["merged"]:
                    for h in range(HEADS):
                        nc.tensor.matmul(atp[:, j, h * 84:(h + 1) * 84],
                                         S["qklin_s"][:, c0:c1],
                                         S["stacked"][h][:, 512:596],
                                         start=True, stop=True)
                else:
                    nc.tensor.matmul(atp[:, j, 0:512],
                                     S["qklin_s"][:, c0:c1],
                                     S["stacked"][p["h"]][:, 0:512],
                                     start=True, stop=True)
            return atp

        def at_pair_ew(p, pi, atp, dest_pair):
            """Paired exp -> per-half min-mult + combine."""
            w = 512 if not p["merged"] else W84
            et = pet.tile([128, 2, 512], BF16, name="et", tag="et")
            nc.scalar.activation(et[:, :, 0:w], atp[:, :, 0:w], AF.Exp)
            for j in range(2):
                ci = 2 * pi + j
                eng = nc.gpsimd if MIN_POOL[ci] else nc.vector
                eng.tensor_scalar(et[:, j, 0:w], et[:, j, 0:w],
                                  csb["ebtab"][:, ci:ci + 1], 1.0,
                                  op0=ALU.mult, op1=ALU.min)
                nc.vector.scalar_tensor_tensor(
                    dest_pair[:, j, 0:w], atp[:, j, 0:w],
                    csb["b1tab"][:, ci:ci + 1],
                    et[:, j, 0:w], op0=ALU.add, op1=ALU.max)

        def at_c4_mm(S, p):
            c0, c1 = CH[4]
            atc = ps_e.tile([128, 512], F32, name="atc", tag="ep")
            if p["merged"]:
                for h in range(HEADS):
                    nc.tensor.matmul(atc[0:84, h * 84:(h + 1) * 84],
                                     S["qklin_s"][:, c0:c1],
                                     S["stacked"][h][:, 512:596],
                                     start=True, stop=True)
            else:
                nc.tensor.matmul(atc[0:84, 0:512], S["qklin_s"][:, c0:c1],
                                 S["stacked"][p["h"]][:, 0:512],
                                 start=True, stop=True)
            return atc

        def at_c4_ew(p, atc, dest):
            w = 512 if not p["merged"] else W84
            et = pet.tile([128, 512], BF16, name="et4", tag="et4")
            nc.scalar.activation(et[0:84, 0:w], atc[0:84, 0:w], AF.Exp)
            eng = nc.gpsimd if MIN_POOL[4] else nc.vector
            eng.tensor_scalar(et[0:84, 0:w], et[0:84, 0:w],
                              csb["ebtab"][0:84, 4:5], 1.0,
                              op0=ALU.mult, op1=ALU.min)
            nc.vector.scalar_tensor_tensor(
                dest[0:84, 0:w], atc[0:84, 0:w], csb["b1tab"][0:84, 4:5],
                et[0:84, 0:w], op0=ALU.add, op1=ALU.max)

        def e_c2(st, c2i):
            S, p, tiles = st["S"], st["p"], st["tiles"]
            c20, c21 = CH[c2i]
            c2sz = c21 - c20
            w = 512 if not p["merged"] else W84
            if c2i == 0:
                st["eps"] = ps_eps.tile([128, 512], F32, name="eps_t", tag="e")
            eps_t = st["eps"]
            a2ps = ps_e.tile([128, 512], F32, name="a2ps", tag="ep")
            for j in range(2):
                nc.tensor.matmul(
                    a2ps[0:128, 0:w],
                    alin_i8[j][:, 256 * c2i:256 * c2i + 256],
                    tiles[j][:, :, 0:w],
                    start=(j == 0), stop=False,
                    perf_mode=mybir.MatmulPerfMode.DoubleRowSwInterleave)
            nc.tensor.matmul(a2ps[0:128, 0:w],
                             alin_bf4[:, 128 * c2i:128 * c2i + 128],
                             tiles[2][0:84, 0:w],
                             start=False, stop=True)
            ext = pext.tile([128, 512], BF16, name="ext", tag="ext")
            nc.scalar.activation(ext[0:c2sz, 0:w], a2ps[0:c2sz, 0:w], AF.Exp,
                                 scale=1.0 / ALSC)
            if p["merged"]:
                # PSUM accumulation groups must not interleave within a
                # bank's zero region: buffer ext tiles, accumulate in e_tail.
                st.setdefault("exts", []).append(ext)
            else:
                nc.tensor.matmul(eps_t[:, 0:512],
                                 S["vtiles"][c2i][0:c2sz,
                                                  p["h"] * 128:
                                                  (p["h"] + 1) * 128],
                                 ext[0:c2sz, 0:512],
                                 start=(c2i == 0), stop=(c2i == 4))

        def e_tail(st):
            """Normalize each head's E by its own softmax denominator
            (times rsV); eall row 64 holds a persistent ones-row that
            carries the lin1 bias through the transposed lin1 stage."""
            S, p, eps_t = st["S"], st["p"], st["eps"]
            w = 512 if not p["merged"] else W84
            eall = S["eall"]
            rsv = S["bca"][0:64, 3:4]
            if p["merged"]:
                for h in range(HEADS):
                    for c2i, (c20, c21) in enumerate(CH):
                        c2sz = c21 - c20
                        nc.tensor.matmul(
                            eps_t[:, h * 84:(h + 1) * 84],
                            S["vtiles"][c2i][0:c2sz, h * 128:(h + 1) * 128],
                            st["exts"][c2i][0:c2sz, h * 84:(h + 1) * 84],
                            start=(c2i == 0), stop=(c2i == 4))
            recip64 = pst.tile([64, 512], F32, name="recip64", tag="recip")
            nc.vector.reciprocal(recip64[:, 0:w], eps_t[64:128, 0:w])
            if p["merged"]:
                for h in range(HEADS):
                    nc.vector.scalar_tensor_tensor(
                        eall[h][0:64, 512:596],
                        eps_t[0:64, h * 84:(h + 1) * 84], rsv,
                        recip64[:, h * 84:(h + 1) * 84],
                        op0=ALU.mult, op1=ALU.mult)
            else:
                h = p["h"]
                nc.vector.scalar_tensor_tensor(
                    eall[h][0:64, 0:512], eps_t[0:64, 0:512], rsv,
                    recip64[:, 0:512], op0=ALU.mult, op1=ALU.mult)

        pending = [None]

        def do_pass(S, p):
            pair0 = pat.tile([128, 2, 512], FP8, name="atp0", tag="atile")
            pair1 = pat.tile([128, 2, 512], FP8, name="atp1", tag="atile")
            at4 = pat.tile([128, 512], BF16, name="at4", tag="a4", bufs=2)
            tiles = [pair0, pair1, at4]
            prev = pending[0]
            # PE: this pass's at matmuls first; Act: prev pass's e-exps
            # flow while the at matmuls run (no head-of-line blocking).
            atp0 = at_pair_mm(S, p, 0)
            if prev is None:
                atp1 = at_pair_mm(S, p, 1)
                atc = at_c4_mm(S, p)
                at_pair_ew(p, 0, atp0, pair0)
                at_pair_ew(p, 1, atp1, pair1)
                at_c4_ew(p, atc, at4)
            else:
                e_c2(prev, 0)
                atp1 = at_pair_mm(S, p, 1)
                e_c2(prev, 1)
                at_pair_ew(p, 0, atp0, pair0)
                e_c2(prev, 2)
                atc = at_c4_mm(S, p)
                e_c2(prev, 3)
                at_pair_ew(p, 1, atp1, pair1)
                e_c2(prev, 4)
                at_c4_ew(p, atc, at4)
                e_tail(prev)
            pending[0] = {"S": S, "p": p, "tiles": tiles}

        def flush_pipe():
            prev = pending[0]
            for c2i in range(5):
                e_c2(prev, c2i)
            e_tail(prev)
            pending[0] = None

        def tail_blk(S, fb):
            """Transposed lin1 for f-block fb: out[f, do] with the softmax
            denominator riding row 64 of the eall copies; normalization via
            a per-partition reciprocal scale in the relu."""
            f0 = fb * 128
            fsz = min(128, N_PIX - f0)
            eall, stats = S["eall"], S["stats128"]
            lps2 = ps_e.tile([128, 64], F32, name="lps2", tag="ep")
            for h in range(HEADS):
                nc.tensor.matmul(lps2[0:fsz, :], eall[h][0:64, f0:f0 + fsz],
                                 lin1w_bf[:, 64 * h:64 * (h + 1)],
                                 start=(h == 0), stop=False)
            nc.tensor.matmul(lps2[0:fsz, :], eall[0][64:65, f0:f0 + fsz],
                             csb["bl1r65"][64:65, :], start=False, stop=True)
            e2t = psq.tile([128, 64], F32, name="e2t", tag="e2t")
            nc.scalar.activation(e2t[0:fsz, :], lps2[0:fsz, :], AF.Relu,
                                 accum_out=stats[0:fsz, fb:fb + 1])
            sq2 = psq.tile([128, 64], F32, name="sq2", tag="sq2")
            nc.gpsimd.tensor_tensor(sq2[0:fsz, :], e2t[0:fsz, :],
                                    e2t[0:fsz, :], op=ALU.mult)
            nc.vector.tensor_reduce(stats[0:fsz, 5 + fb:6 + fb],
                                    sq2[0:fsz, :],
                                    axis=mybir.AxisListType.X, op=ALU.add)
            nc.gpsimd.tensor_reduce(S["pm"][0:1, fb, :], e2t[0:fsz, :],
                                    axis=mybir.AxisListType.C, op=ALU.max)

        def tail_fin(S):
            """Combine per-block stats, LN2 scalars, normalized max-pool."""
            s, stats = S["s"], S["stats128"]
            st10_ps = ps_e.tile([1, 10], F32, name="st10_ps", tag="ep")
            nc.tensor.matmul(st10_ps[0:1, :], csb["ones_c"][:, 0:1],
                             stats[:, :], start=True, stop=True)
            ls2 = pst.tile([1, 2], F32, name="ls2", tag="ls2")
            nc.vector.tensor_reduce(
                ls2[:, :].rearrange("p (a u) -> p a u", u=1),
                st10_ps[0:1, :].rearrange("p (a b) -> p a b", a=2),
                axis=mybir.AxisListType.X, op=ALU.add)
            emax_do = pst.tile([1, 64], F32, name="emax_do", tag="emax_do")
            nc.vector.tensor_reduce(
                emax_do[0:1, :].rearrange("p (b u) -> p b u", u=1),
                S["pm"][0:1, :, :].rearrange("p a b -> p b a"),
                axis=mybir.AxisListType.X, op=ALU.max)
            emt_ps = ps_e.tile([64, 1], F32, name="emt_ps", tag="ep")
            nc.tensor.transpose(emt_ps[:, :], emax_do[0:1, :],
                                csb["ones_c"][0:1, 0:1])
            # per-sample LN2 scalars (mean/var -> Newton rsqrt)
            m2 = pst.tile([1, 2], F32, name="m2", tag="m2")
            nc.vector.tensor_scalar_mul(m2[:, :], ls2[:, :], 1.0 / LN2_N)
            ve = pst.tile([1, 2], F32, name="ve", tag="ve")
            nc.vector.scalar_tensor_tensor(ve[:, 1:2], m2[:, 0:1], -1.0,
                                           m2[:, 0:1],
                                           op0=ALU.mult, op1=ALU.mult)
            nc.vector.scalar_tensor_tensor(ve[:, 0:1], m2[:, 1:2], EPS,
                                           ve[:, 1:2], op0=ALU.add,
                                           op1=ALU.add)
            yk2 = pst.tile([1, 1], I32, name="yk2", tag="yk2")
            nc.vector.tensor_scalar(yk2[:, :], ve.bitcast(I32)[:, 0:1],
                                    1, None, op0=ALU.logical_shift_right)
            nc.vector.tensor_scalar(yk2[:, :], yk2[:, :], -1, RSQ_MAGIC,
                                    op0=ALU.mult, op1=ALU.add)
            y2 = yk2.bitcast(F32)
            nt2 = pst.tile([1, 1], F32, name="nt2", tag="nt2")
            for _ in range(2):
                nc.vector.tensor_tensor(nt2[:, :], y2[:, :], y2[:, :],
                                        op=ALU.mult)
                nc.vector.tensor_tensor(nt2[:, :], nt2[:, :], ve[:, 0:1],
                                        op=ALU.mult)
                nc.vector.tensor_scalar(nt2[:, :], nt2[:, :], -0.5, 1.5,
                                        op0=ALU.mult, op1=ALU.add)
                nc.vector.tensor_tensor(y2[:, :], y2[:, :], nt2[:, :],
                                        op=ALU.mult)
            rsnm = pst.tile([1, 2], F32, name="rsnm", tag="rsnm")
            nc.vector.tensor_copy(rsnm[:, 0:1], y2[:, :])
            nc.vector.scalar_tensor_tensor(rsnm[:, 1:2], m2[:, 0:1], -1.0,
                                           y2[:, :], op0=ALU.mult,
                                           op1=ALU.mult)
            bc2_ps = ps_e.tile([64, 2], F32, name="bc2_ps", tag="ep")
            nc.tensor.matmul(bc2_ps[:, :], csb["ones_r"][0:1, 0:64],
                             rsnm[:, :], start=True, stop=True)
            bc2s = pst.tile([64, 2], F32, name="bc2s", tag="bc2s")
            nc.vector.tensor_copy(bc2s[:, :], bc2_ps[:, :])
            emt = pst.tile([64, 1], F32, name="emt", tag="emt")
            nc.vector.tensor_copy(emt[:, :], emt_ps[:, :])
            nc.vector.tensor_scalar(emax_all[:, s:s + 1], emt[:, :],
                                    bc2s[:, 0:1], bc2s[:, 1:2],
                                    op0=ALU.mult, op1=ALU.add)

        # ---- pipelined schedule: sample s+1's front-end is emitted between
        # sample s's attention passes; the at/e pass pipeline is carried
        # across the sample boundary.
        S = front_a(0)
        front_b(S)
        front_c1(S)
        front_c2(S)
        states = {0: S}
        for s in range(spb):
            S = states[s]
            plist = ([dict(h=h, merged=False) for h in range(HEADS)]
                     + [dict(h=None, merged=True)])
            do_pass(S, plist[0])
            if s > 0:
                Sp = states.pop(s - 1)
                tail_blk(Sp, 4)
                tail_fin(Sp)
            if s + 1 < spb:
                Sn = front_a(s + 1)
            do_pass(S, plist[1])
            if s + 1 < spb:
                front_b(Sn)
            do_pass(S, plist[2])
            if s + 1 < spb:
                front_c1(Sn)
            do_pass(S, plist[3])
            if s + 1 < spb:
                front_c2(Sn)
                states[s + 1] = Sn
            do_pass(S, plist[4])
            for fb in range(4):
                tail_blk(S, fb)
        flush_pipe()
        Sp = states.pop(spb - 1)
        tail_blk(Sp, 4)
        tail_fin(Sp)

        # ---------------- lin2 + final elu ----------------
        l2ps = ps_e.tile([10, spb], F32, name="l2ps", tag="ep")
        nc.tensor.matmul(l2ps[:, :], csb["lin2w"][:, :], emax_all[:, :],
                         start=True, stop=True)
        fe = pst.tile([10, spb], F32, name="fe", tag="fe")
        nc.scalar.activation(fe[:, :], l2ps[:, :], AF.Exp,
                             bias=csb["bl2"][:, 0:1])
        nc.vector.tensor_scalar(fe[:, :], fe[:, :], 1.0, -1.0,
                                op0=ALU.min, op1=ALU.add)
        out_sb = pst.tile([10, spb], F32, name="out_sb", tag="out_sb")
        nc.vector.scalar_tensor_tensor(out_sb[:, :], l2ps[:, :],
                                       csb["bl2"][:, 0:1], fe[:, :],
                                       op0=ALU.add, op1=ALU.max)
        nc.sync.dma_start(out=out_dram.rearrange("s t -> t s"), in_=out_sb[:, :])

    return nc


def _reference_numpy(inp):
    """Pure-numpy fallback (only used if LN affine params are nontrivial)."""
    def ln(x, g=None, b=None):
        axes = tuple(range(1, x.ndim))
        mu = x.mean(axis=axes, keepdims=True)
        var = x.var(axis=axes, keepdims=True)
        y = (x - mu) / np.sqrt(var + EPS)
        return y * g + b if g is not None else y

    def elu(x):
        return np.where(x > 0, x, np.expm1(np.minimum(x, 0)))

    x = np.asarray(inp["x"], np.float64)
    N = x.shape[0]
    w1, b1 = np.asarray(inp["conv1_w"], np.float64), np.asarray(inp["conv1_b"], np.float64)
    h = np.zeros((N, 16, 150, 5))
    for di in range(2):
        for dj in range(2):
            h += np.einsum("oc,nchw->nohw", w1[:, :, di, dj],
                           x[:, :, di:di + 150, dj:dj + 5])
    h = np.maximum(h + b1[None, :, None, None], 0)
    w2, b2 = np.asarray(inp["conv2_w"], np.float64), np.asarray(inp["conv2_b"], np.float64)
    h2 = np.zeros((N, 32, 149, 4))
    for di in range(2):
        for dj in range(2):
            h2 += np.einsum("oc,nchw->nohw", w2[:, :, di, dj],
                            h[:, :, di:di + 149, dj:dj + 4])
    h2 = np.maximum(h2 + b2[None, :, None, None], 0)
    p = np.arange(N_PIX)
    xc, yc = (p % 4) / 4.0, (p // 4) / 149.0
    feats = np.concatenate(
        [h2.transpose(0, 2, 3, 1).reshape(N, N_PIX, 32),
         np.broadcast_to(np.stack([xc, yc], 1)[None], (N, N_PIX, 2))], axis=2)

    def proj(wn, bn, gn, bn2):
        P = (feats @ np.asarray(inp[wn], np.float64) + np.asarray(inp[bn], np.float64))
        P = P.reshape(N, N_PIX, HEADS, D).transpose(0, 2, 1, 3)
        return ln(P, np.asarray(inp[gn], np.float64), np.asarray(inp[bn2], np.float64))

    K = proj("kp_w", "kp_b", "knorm_g", "knorm_b")
    Q = proj("qp_w", "qp_b", "qnorm_g", "qnorm_b")
    V = proj("vp_w", "vp_b", "vnorm_g", "vnorm_b")
    A = elu(Q @ np.asarray(inp["qlin_w"], np.float64) + np.asarray(inp["qlin_b"], np.float64)
            + K @ np.asarray(inp["klin_w"], np.float64) + np.asarray(inp["klin_b"], np.float64))
    A = A @ np.asarray(inp["alin_w"], np.float64) + np.asarray(inp["alin_b"], np.float64)
    A = A - A.max(axis=-1, keepdims=True)
    A = np.exp(A)
    A = A / A.sum(axis=-1, keepdims=True)
    E = np.einsum("bhfc,bhcd->bhfd", A, V)
    E = E.transpose(0, 2, 1, 3).reshape(N, N_PIX, HEADS * D)
    E = np.maximum(E @ np.asarray(inp["lin1_w"], np.float64)
                   + np.asarray(inp["lin1_b"], np.float64), 0)
    E = ln(E)
    E = E.max(axis=1)
    out = E @ np.asarray(inp["lin2_w"], np.float64) + np.asarray(inp["lin2_b"], np.float64)
    return elu(out).astype(np.float32)


def kernel(**inputs):
    trivial = (np.all(np.asarray(inputs["knorm_g"]) == 1.0)
               and np.all(np.asarray(inputs["knorm_b"]) == 0.0)
               and np.all(np.asarray(inputs["qnorm_g"]) == 1.0)
               and np.all(np.asarray(inputs["qnorm_b"]) == 0.0)
               and np.all(np.asarray(inputs["vnorm_g"]) == 1.0)
               and np.all(np.asarray(inputs["vnorm_b"]) == 0.0))
    if not trivial:
        return _reference_numpy(inputs)

    x = np.ascontiguousarray(np.asarray(inputs["x"], np.float32))
    n = x.shape[0]
    assert n == N_CORES * SPB, f"expected batch {N_CORES * SPB}, got {n}"
    consts = _prep_consts(inputs)

    if "nc" not in _cache:
        nc = build_nc(SPB)
        nc.compile()
        _cache["nc"] = nc
    nc = _cache["nc"]

    in_maps = []
    for c in range(N_CORES):
        m = dict(consts)
        m["x"] = np.ascontiguousarray(x[c * SPB:(c + 1) * SPB])
        in_maps.append(m)

    import os
    trace = bool(int(os.environ.get("KERNEL_TRACE", "0")))
    res = run_bass_kernel_spmd(nc, in_maps, list(range(N_CORES)), trace=trace)
    kernel._last_results = res
    out = np.concatenate([np.asarray(r["out"]) for r in res.results], axis=0)
    return out.astype(np.float32)


kernel._last_results = None


# revision 52
# speedup vs baseline: 1.0729x; 1.0613x over previous
"""Fused Trainium2 kernel for nn_MultiHeadRelationalModule.

Data-parallel over 8 NeuronCores (8 samples each). The whole per-sample
pipeline (conv1 -> conv2 -> +coords -> K/Q/V proj -> LayerNorm ->
relational attention (4 heads, 596x596) -> softmax -> weighted sum ->
lin1 -> LN -> maxpool -> lin2 -> elu) runs on-chip; the big attention
maps never touch HBM.

v3 engine-balance rework (vs v2 baseline):
  * Act engine runs ONLY Exp/Ln/Relu-family functions (all in the
    natural_log_exp_and_others table) -> a single act-table load for the
    whole kernel (was 19 loads = 24us).
  * LN rsqrt computed as exp(-0.5*ln(var+eps)) on Act (no Sqrt table).
  * at-stage exp is bias-free: exp(z+b) = exp(z)*exp(b); exp(b) folds
    into the following min-op scalar (DVE 4x bf16 mode: 0.26ns/elem).
    Bias-free exp reads PAIRED 2-bank PSUM tiles (half the Act ops).
  * softmax-exp bias exp(alin_b - colsum(alin_w)) folds multiplicatively
    into the V tiles (and their denominator ones-columns), so the second
    exp is also bias-free.
  * Q/K/V projection biases ride a 35th ones-row of feats through the
    projection matmuls; LN means fold into the post-matmul adds; LN
    scales fold into a per-sample copy of qlin/klin rows (rsQ/rsK) and
    into the softmax-normalize multiply (rsV).
  * elu combine ops split across DVE and Pool (Pool reads PSUM fine);
    conv relus + V builds + misc moved to Pool; tail LN stats via
    tensor_scalar accum / tensor_tensor_reduce on DVE.

Key identities:
  elu(x)+1 == max(x + 1, min(exp(x), 1))          (exact)
  A' = elu(z)+1 fed to matmul with alin_w: subtract colsum(alin_w) in
       the softmax bias to undo the +1; that bias is then moved out of
       the exp into a multiplicative row-scale on V.
  max-pool commutes with the final LN (monotone affine map).
"""

import numpy as np
from contextlib import ExitStack

import concourse.bacc as bacc
import concourse.bass as bass
import concourse.mybir as mybir
import concourse.tile as tile
from concourse.bass_utils import run_bass_kernel_spmd

F32 = mybir.dt.float32
BF16 = mybir.dt.bfloat16
FP8 = mybir.dt.float8e4
I32 = mybir.dt.int32
RSQ_MAGIC = 0x5F3759DF
ALSC = 16.0  # alin pre-scale into fp8e4m3 normal range; undone in exp scale
AF = mybir.ActivationFunctionType
ALU = mybir.AluOpType

N_CORES = 8
SPB = 8               # samples per core
N_PIX = 596
HEADS = 4
D = 64
CH = [(0, 128), (128, 256), (256, 384), (384, 512), (512, 596)]
FH = [(0, 512), (512, 596)]
SHIFTS = [(0, 0), (0, 1), (1, 0), (1, 1)]
LN_N = float(HEADS * N_PIX * D)       # 152576
LN2_N = float(N_PIX * D)              # 38144
EPS = 1e-5
W84 = 84 * HEADS

_cache = {}


def _prep_consts(inp):
    """Host-side preprocessing of weights into kernel-friendly layouts."""
    f = np.float32
    c = {}
    conv1_w = np.asarray(inp["conv1_w"], f)
    c["w1s"] = np.ascontiguousarray(
        np.concatenate([conv1_w[:, :, di, dj].T for (di, dj) in SHIFTS], axis=1)
    )  # (4, 64)
    c["b1"] = np.ascontiguousarray(np.asarray(inp["conv1_b"], f)[:, None])  # (16,1)
    conv2_w = np.asarray(inp["conv2_w"], f)
    c["w2s"] = np.ascontiguousarray(
        np.concatenate([conv2_w[:, :, di, dj].T for (di, dj) in SHIFTS], axis=1)
    )  # (16, 128)
    c["b2"] = np.ascontiguousarray(np.asarray(inp["conv2_b"], f)[:, None])  # (32,1)

    p = np.arange(N_PIX)
    c["coords3"] = np.ascontiguousarray(
        np.stack([(p % 4) / 4.0, (p // 4) / 149.0,
                  np.ones(N_PIX)]).astype(f)
    )  # (3, 596): xc, yc, ones-row (projection bias carrier)

    # Q/K projection merged per head with bias in a 35th feats-ones row:
    # cols h*128:h*128+64 = Q, cols h*128+64:h*128+128 = K.
    qp_w = np.asarray(inp["qp_w"], f)
    kp_w = np.asarray(inp["kp_w"], f)
    qp_b = np.asarray(inp["qp_b"], f)
    kp_b = np.asarray(inp["kp_b"], f)
    kqw2e = np.zeros((35, 512), f)
    for h in range(HEADS):
        kqw2e[0:34, h * 128:h * 128 + 64] = qp_w[:, h * 64:(h + 1) * 64]
        kqw2e[0:34, h * 128 + 64:h * 128 + 128] = kp_w[:, h * 64:(h + 1) * 64]
        kqw2e[34, h * 128:h * 128 + 64] = qp_b[h * 64:(h + 1) * 64]
        kqw2e[34, h * 128 + 64:h * 128 + 128] = kp_b[h * 64:(h + 1) * 64]
    c["kqw2e"] = kqw2e

    vwe = np.zeros((35, 256), f)
    vwe[0:34] = np.asarray(inp["vp_w"], f)
    vwe[34] = np.asarray(inp["vp_b"], f)
    c["vwe"] = vwe

    c["qklin"] = np.ascontiguousarray(
        np.concatenate([np.asarray(inp["qlin_w"], f),
                        np.asarray(inp["klin_w"], f)], axis=0)
    )  # (128, 596): rows 0:64 qlin (Q), 64:128 klin (K)

    qkl_b = np.asarray(inp["qlin_b"], f) + np.asarray(inp["klin_b"], f)
    b1tab = np.zeros((128, 5), f)
    ebtab = np.zeros((128, 5), f)
    for ci, (c0, c1) in enumerate(CH):
        b1tab[0:c1 - c0, ci] = qkl_b[c0:c1] + 1.0
        ebtab[0:c1 - c0, ci] = np.exp(qkl_b[c0:c1].astype(np.float64)).astype(f)
    c["b1tab"] = b1tab
    c["ebtab"] = ebtab

    # fp8e4m3 DoubleRowSwInterleave weight pairs for alin rows 0:512 (x16 so
    # the ~0.05-scale entries sit in e4m3's normal range; undone in exp scale).
    import ml_dtypes
    alin_w = np.asarray(inp["alin_w"], f)
    alin16 = np.pad(alin_w * ALSC, ((0, 0), (0, 44)))
    for j in range(2):
        A = alin16[256 * j:256 * j + 128]
        B = alin16[256 * j + 128:256 * j + 256]
        buf = np.zeros((128, 1280), f)
        for ci in range(5):
            c0 = 128 * ci
            blk = np.empty((128, 256), f)
            blk[:, 0::2] = A[:, c0:c0 + 128][:, ::-1]
            blk[:, 1::2] = B[:, c0:c0 + 128][:, ::-1]
            buf[:, 2 * c0:2 * c0 + 256] = blk
        c[f"alin_i8_{j}"] = np.ascontiguousarray(
            buf.astype(ml_dtypes.float8_e4m3))
    c["alin4"] = np.ascontiguousarray(
        np.pad(alin_w[512:596, :] * ALSC, ((0, 0), (0, 44))
               ).astype(ml_dtypes.bfloat16))  # (84, 640), pre-scaled

    # softmax bias exp(alin_b - colsum(alin_w)) folded into V rows (c2 dim)
    s = np.exp((np.asarray(inp["alin_b"], np.float64)
                - np.asarray(inp["alin_w"], np.float64).sum(axis=0)))
    s = s.astype(f)
    s_cols = np.zeros((128, 5), f)
    for ci, (c0, c1) in enumerate(CH):
        s_cols[0:c1 - c0, ci] = s[c0:c1]
    c["s_cols"] = s_cols

    lin1_w = np.asarray(inp["lin1_w"], f)
    l1 = np.zeros((64, 256), f)
    for h in range(HEADS):
        l1[:, h * 64:(h + 1) * 64] = lin1_w[h * 64:(h + 1) * 64, :]
    c["lin1w"] = l1
    c["bl1"] = np.ascontiguousarray(np.asarray(inp["lin1_b"], f)[:, None])  # (64,1)
    b65 = np.zeros((65, 64), f)
    b65[64, :] = np.asarray(inp["lin1_b"], f)
    c["bl1r65"] = np.ascontiguousarray(b65.astype(ml_dtypes.bfloat16))
    c["lin2w"] = np.ascontiguousarray(np.asarray(inp["lin2_w"], f))  # (64,10)
    bl2 = np.zeros((10, 2), f)
    bl2[:, 0] = np.asarray(inp["lin2_b"], f)
    bl2[:, 1] = np.asarray(inp["lin2_b"], f) + 1.0
    c["bl2"] = bl2
    c["ones_r"] = np.ones((1, 128), f)
    c["ones_c"] = np.ones((128, 1), f)
    c["epsc"] = np.full((1, 1), EPS, f)
    c["id35"] = np.eye(35, dtype=f)
    # row-group selectors: cols of mm(sel, t62) pick (-mu, rs) per partition
    selqk3 = np.zeros((3, 128), f)
    selqk3[0, 0:64] = 1.0
    selqk3[1, 64:128] = 1.0
    selv3 = np.zeros((3, 128), f)
    selv3[2, :] = 1.0
    c["selqk3"] = selqk3
    c["selv3"] = selv3
    # LN-stat helpers: per tensor T with extended weights W' (35, .):
    # sum(T) = s'^T W' 1 and ssq(T) = <G, W' W'^T> with G = feats' feats'^T
    # (biases included via the ones-row of feats').
    wq = np.concatenate([qp_w, qp_b[None]], 0)
    wk = np.concatenate([kp_w, kp_b[None]], 0)
    wsum3 = np.zeros((35, 3), f)
    wsum3[:, 0] = wq.sum(axis=1)
    wsum3[:, 1] = wk.sum(axis=1)
    wsum3[:, 2] = vwe.sum(axis=1)
    c["wsum3"] = wsum3.astype(ml_dtypes.bfloat16)
    for k in ("w1s", "w2s", "coords3", "kqw2e", "vwe", "qklin", "lin1w",
              "id35"):
        c[k] = c[k].astype(ml_dtypes.bfloat16)
    c["gmq"] = np.ascontiguousarray((wq @ wq.T).astype(f))   # (35, 35)
    c["gmk"] = np.ascontiguousarray((wk @ wk.T).astype(f))
    c["gmv"] = np.ascontiguousarray((vwe @ vwe.T).astype(f))
    return c


CONST_SHAPES = {
    "b1": (16, 1), "b2": (32, 1),
    "b1tab": (128, 5), "ebtab": (128, 5), "s_cols": (128, 5),
    "bl1": (64, 1), "lin2w": (64, 10),
    "bl2": (10, 2), "ones_r": (1, 128), "ones_c": (128, 1), "epsc": (1, 1),
    "selqk3": (3, 128), "selv3": (3, 128),
    "gmq": (35, 35), "gmk": (35, 35), "gmv": (35, 35),
}
CONST_BF16 = {
    "bl1r65": (65, 64),
    "w1s": (4, 64), "w2s": (16, 128), "coords3": (3, N_PIX),
    "kqw2e": (35, 512), "vwe": (35, 256), "qklin": (128, N_PIX),
    "lin1w": (64, 256), "alin4": (84, 640), "wsum3": (35, 3),
    "id35": (35, 35),
}
CONST_FP8 = {"alin_i8_0": (128, 1280), "alin_i8_1": (128, 1280)}

# elu min-op engine split per chunk: True -> Pool, False -> DVE (4x)
MIN_POOL = [False, False, False, False, False]


def build_nc(spb=SPB):
    """Build the Bass program (same program runs SPMD on each core)."""
    nc = bacc.Bacc("TRN2", target_bir_lowering=False, debug=False)

    x_dram = nc.dram_tensor("x", [spb, 4, 151, 6], F32, kind="ExternalInput").ap()
    out_dram = nc.dram_tensor("out", [spb, 10], F32, kind="ExternalOutput").ap()
    cdram = {
        k: nc.dram_tensor(k, list(v), F32, kind="ExternalInput").ap()
        for k, v in CONST_SHAPES.items()
    }
    for k, v in CONST_BF16.items():
        cdram[k] = nc.dram_tensor(k, list(v), BF16, kind="ExternalInput").ap()
    for k, v in CONST_FP8.items():
        cdram[k] = nc.dram_tensor(k, list(v), FP8, kind="ExternalInput").ap()

    with tile.TileContext(nc) as tc, ExitStack() as ctx:
        pc = ctx.enter_context(tc.tile_pool(name="consts", bufs=1))
        # SBUF pools
        px = ctx.enter_context(tc.tile_pool(name="px", bufs=2))
        ph1 = ctx.enter_context(tc.tile_pool(name="ph1", bufs=2))
        pfeat = ctx.enter_context(tc.tile_pool(name="pfeat", bufs=2))
        pstk = ctx.enter_context(tc.tile_pool(name="pstk", bufs=8))
        pqs = ctx.enter_context(tc.tile_pool(name="pqs", bufs=2))
        pbc = ctx.enter_context(tc.tile_pool(name="pbc", bufs=2))
        pv = ctx.enter_context(tc.tile_pool(name="pv", bufs=10))
        pet = ctx.enter_context(tc.tile_pool(name="pet", bufs=3))
        pat = ctx.enter_context(tc.tile_pool(name="pat", bufs=4))
        pext = ctx.enter_context(tc.tile_pool(name="pext", bufs=7))
        psq = ctx.enter_context(tc.tile_pool(name="psq", bufs=2))
        pst = ctx.enter_context(tc.tile_pool(name="pst", bufs=3))
        peall = ctx.enter_context(tc.tile_pool(name="peall", bufs=8))
        ptl = ctx.enter_context(tc.tile_pool(name="ptl", bufs=2))
        pfix = ctx.enter_context(tc.tile_pool(name="pfix", bufs=1))
        # PSUM pools: 8 banks = at-pairs 2x2 + e-ring 2 + eps 1 + fr 1.
        PS = bass.MemorySpace.PSUM
        ps_atp = ctx.enter_context(tc.tile_pool(name="ps_atp", bufs=2, space=PS))
        ps_e = ctx.enter_context(tc.tile_pool(name="ps_e", bufs=2, space=PS))
        ps_eps = ctx.enter_context(tc.tile_pool(name="ps_eps", bufs=1, space=PS))
        ps_fr = ctx.enter_context(tc.tile_pool(name="ps_fr", bufs=1, space=PS))

        # ---- prefetch sample 0's input before the const DMAs ----
        x_t0 = px.tile([4, 151, 6], F32, name="x_t", tag="x")
        nc.sync.dma_start(out=x_t0[:, :, :], in_=x_dram[0])

        # ---- load constants; critical-path consts first ----
        csb = {}
        first = ["w1s", "b1", "w2s", "b2", "coords3", "kqw2e", "vwe",
                 "qklin", "wsum3", "id35", "selqk3", "selv3", "s_cols",
                 "b1tab", "ebtab", "gmq", "gmk", "gmv"]
        order = first + [k for k in list(CONST_SHAPES) + list(CONST_BF16)
                         if k not in first]
        dmaq = [nc.sync, nc.scalar, nc.gpsimd]
        for i, k in enumerate(order):
            if k in CONST_SHAPES:
                shp, dt = CONST_SHAPES[k], F32
            else:
                shp, dt = CONST_BF16[k], BF16
            t = pc.tile(list(shp), dt, name=f"c_{k}")
            dmaq[i % 3].dma_start(out=t[:, :], in_=cdram[k][:, :])
            csb[k] = t
        alin_i8 = []
        for j in range(2):
            t = pc.tile([128, 1280], FP8, name=f"alin_i8_{j}")
            dmaq[j].dma_start(out=t[:, :], in_=cdram[f"alin_i8_{j}"][:, :])
            alin_i8.append(t)

        w1s_bf = csb["w1s"]
        w2s_bf = csb["w2s"]
        kqw2e_bf = csb["kqw2e"]
        vwe_bf = csb["vwe"]
        qklin_bf = csb["qklin"]
        lin1w_bf = csb["lin1w"]
        id35_bf = csb["id35"]
        alin_bf4 = csb["alin4"]
        wsum3_bf = csb["wsum3"]
        ones_bf = pc.tile([128, 1], BF16, name="ones_bf")
        nc.vector.memset(ones_bf[:, :], 1.0)
        # sB: per-c2-chunk softmax scale broadcast, built from s_cols
        ones256 = pc.tile([128, 256], BF16, name="ones256")
        nc.vector.memset(ones256[:, :], 1.0)
        sB_bf = pc.tile([128, 5, 256], BF16, name="sB_bf")
        for ci in range(5):
            nc.vector.tensor_scalar_mul(sB_bf[:, ci, :], ones256[:, :],
                                        csb["s_cols"][:, ci:ci + 1])
        sB3 = sB_bf
        # feats'-transpose staging tiles; col 35 is a persistent ones column
        # so the Gram matmul also yields the feature sums s'.
        ft_bufs = []
        for i in range(3):
            fb = pst.tile([128, 36], BF16, name="ft_sb", tag="ft")
            nc.vector.memset(fb[:, 35:36], 1.0)
            ft_bufs.append(fb)
        emax_all = pfix.tile([64, spb], F32, name="emax_all")

        # eall ring: row 64 is a persistent ones-row (bias carrier for the
        # transposed lin1); the per-pass normalize writes only rows 0:64.
        for i in range(8):
            eb_t = peall.tile([65, N_PIX], BF16, name="eall_i", tag="eall")
            nc.vector.memset(eb_t[64:65, :], 1.0)

        # feats tiles: conv writes rows 0:32; rows 32:34 coords, row 34 ones,
        # both persistent (written once into each ring buffer).
        feats_bufs = []
        for i in range(2):
            ft = pfeat.tile([35, N_PIX], BF16, name="feats", tag="feats")
            nc.vector.tensor_copy(ft[32:35, :], csb["coords3"][:, :])
            feats_bufs.append(ft)

        # V tiles: cols h*128+64:h*128+128 hold the persistent softmax-scale
        # block s[c2] (denominator ones-columns, pre-scaled).
        for i in range(10):
            vt = pv.tile([128, 512], BF16, name="vt", tag="v")
            vt3 = vt.rearrange("p (h c) -> p h c", c=128)
            ci = i % 5
            csz = CH[ci][1] - CH[ci][0]
            nc.vector.tensor_copy(
                vt3[0:csz, :, 64:128],
                sB3[0:csz, ci, :].rearrange("p (h d) -> p h d", h=4))

        # ================= pipelined per-sample stages =================

        def front_a(s):
            """x load/cast + conv1 + conv2 -> feats (relu on Pool)."""
            S = {"s": s}
            if s == 0:
                x_t = x_t0
            else:
                x_t = px.tile([4, 151, 6], F32, name="x_t", tag="x")
                nc.sync.dma_start(out=x_t[:, :, :], in_=x_dram[s])
            x_bf = px.tile([4, 151, 6], BF16, name="x_bf", tag="xbf")
            nc.gpsimd.tensor_copy(x_bf[:, :, :], x_t[:, :, :])

            h1 = ph1.tile([16, 750], BF16, name="h1", tag="h1")
            h1v = h1.rearrange("c (h w) -> c h w", w=5)
            for (r0, nr, dst0) in ((0, 102, 0), (102, 48, 510)):
                cps = ps_fr.tile([16, nr * 5], F32, name="c1ps", tag="fr")
                for si, (di, dj) in enumerate(SHIFTS):
                    nc.tensor.matmul(
                        cps[:, :],
                        w1s_bf[:, si * 16:(si + 1) * 16],
                        x_bf[:, di + r0:di + r0 + nr, dj:dj + 5],
                        start=(si == 0), stop=(si == 3),
                    )
                nc.vector.tensor_scalar(h1[:, dst0:dst0 + nr * 5], cps[:, :],
                                        csb["b1"][:, 0:1], 0.0,
                                        op0=ALU.add, op1=ALU.max)

            feats = feats_bufs[s % 2]
            for (r0, nr, dst0) in ((0, 128, 0), (128, 21, 512)):
                cps = ps_fr.tile([32, nr * 4], F32, name="c2ps", tag="fr")
                for si, (di, dj) in enumerate(SHIFTS):
                    nc.tensor.matmul(
                        cps[:, :],
                        w2s_bf[:, si * 32:(si + 1) * 32],
                        h1v[:, di + r0:di + r0 + nr, dj:dj + 4],
                        start=(si == 0), stop=(si == 3),
                    )
                nc.vector.tensor_scalar(feats[0:32, dst0:dst0 + nr * 4],
                                        cps[:, :], csb["b2"][:, 0:1], 0.0,
                                        op0=ALU.add, op1=ALU.max)
            S["feats"] = feats
            return S

        def front_b(S):
            """LN stats: G36 = [feats'|1]^T-gram on the PE (last col = s'),
            then ssq = <G, W W^T> via ttr against host Gram mats."""
            feats = S["feats"]
            g_ps = ps_fr.tile([36, 36], F32, name="g_ps", tag="fr")
            for ci, (c0, c1) in enumerate(CH):
                csz = c1 - c0
                ft_ps = ps_e.tile([128, 35], BF16, name="ft_ps", tag="ep")
                nc.tensor.transpose(ft_ps[0:csz, :], feats[:, c0:c1],
                                    id35_bf[:, :])
                ft_sb = ft_bufs[ci % 3]
                nc.vector.tensor_copy(ft_sb[0:csz, 0:35], ft_ps[0:csz, :])
                nc.tensor.matmul(g_ps[:, :], ft_sb[0:csz, :],
                                 ft_sb[0:csz, :],
                                 start=(ci == 0), stop=(ci == 4))
            g_sb = pst.tile([36, 36], BF16, name="g_sb", tag="g_sb")
            nc.vector.tensor_copy(g_sb[:, :], g_ps[:, :])
            gw = psq.tile([35, 3, 35], F32, name="gw", tag="gw")
            acc3 = pst.tile([35, 3], F32, name="acc3", tag="acc3")
            for i, gm in enumerate(("gmq", "gmk", "gmv")):
                nc.gpsimd.tensor_tensor(gw[:, i, :], g_sb[0:35, 0:35],
                                        csb[gm][:, :], op=ALU.mult)
            nc.vector.tensor_reduce(
                acc3[:, :].rearrange("p (a u) -> p a u", u=1),
                gw[:, :, :], axis=mybir.AxisListType.X, op=ALU.add)
            stats_ps = ps_fr.tile([1, 6], F32, name="stats_ps", tag="fr")
            nc.tensor.matmul(stats_ps[0:1, 0:3], g_sb[0:35, 35:36],
                             wsum3_bf[:, :], start=True, stop=True)
            nc.tensor.matmul(stats_ps[0:1, 3:6], csb["ones_c"][0:35, 0:1],
                             acc3[:, :], start=True, stop=True)
            mu3 = pst.tile([1, 3], F32, name="mu3", tag="mu3")
            nc.vector.tensor_scalar_mul(mu3[:, :], stats_ps[0:1, 0:3],
                                        1.0 / LN_N)
            msq3 = pst.tile([1, 3], F32, name="msq3", tag="msq3")
            nc.vector.tensor_scalar_mul(msq3[:, :], stats_ps[0:1, 3:6],
                                        1.0 / LN_N)
            S["mu3"] = mu3
            S["msq3"] = msq3
            return S

        def front_c1(S):
            """LN scalars via Newton rsqrt on DVE."""
            mu3, msq3 = S["mu3"], S["msq3"]
            nmu2 = pst.tile([1, 3], F32, name="nmu2", tag="nmu2")
            nc.vector.scalar_tensor_tensor(nmu2[:, :], mu3[:, :], -1.0,
                                           mu3[:, :],
                                           op0=ALU.mult, op1=ALU.mult)
            var3e = pst.tile([1, 3], F32, name="var3e", tag="var3e")
            nc.vector.scalar_tensor_tensor(var3e[:, :], msq3[:, :], EPS,
                                           nmu2[:, :], op0=ALU.add,
                                           op1=ALU.add)
            nm3 = pst.tile([1, 3], F32, name="nm3", tag="nm3")
            nc.vector.tensor_scalar_mul(nm3[:, :], mu3[:, :], -1.0)
            # transpose (-mu | var) onto partitions 0:3
            t6_ps = ps_fr.tile([3, 2], F32, name="t6_ps", tag="fr")
            nc.tensor.transpose(t6_ps[:, 0:1], nm3[0:1, :],
                                csb["ones_c"][0:1, 0:1])
            nc.tensor.transpose(t6_ps[:, 1:2], var3e[0:1, :],
                                csb["ones_c"][0:1, 0:1])
            t62 = pst.tile([3, 2], F32, name="t62", tag="t62")
            nc.vector.tensor_copy(t62[:, :], t6_ps[:, :])
            # fast inverse sqrt + 2 Newton steps: rs = (var+eps)^-0.5
            yk = pst.tile([3, 1], I32, name="yk", tag="yk")
            nc.vector.tensor_scalar(yk[:, :], t62.bitcast(I32)[:, 1:2],
                                    1, None, op0=ALU.logical_shift_right)
            nc.vector.tensor_scalar(yk[:, :], yk[:, :], -1, RSQ_MAGIC,
                                    op0=ALU.mult, op1=ALU.add)
            y = yk.bitcast(F32)
            nt = pst.tile([3, 1], F32, name="nt", tag="nt")
            for _ in range(2):
                nc.vector.tensor_tensor(nt[:, :], y[:, :], y[:, :],
                                        op=ALU.mult)
                nc.vector.tensor_tensor(nt[:, :], nt[:, :], t62[:, 1:2],
                                        op=ALU.mult)
                nc.vector.tensor_scalar(nt[:, :], nt[:, :], -0.5, 1.5,
                                        op0=ALU.mult, op1=ALU.add)
                nc.vector.tensor_tensor(y[:, :], y[:, :], nt[:, :],
                                        op=ALU.mult)
            nc.vector.tensor_copy(t62[:, 1:2], y[:, :])
            # broadcast (-mu, rs) to per-partition columns via row selectors
            bca_ps = ps_fr.tile([128, 4], F32, name="bca_ps", tag="fr")
            nc.tensor.matmul(bca_ps[:, 0:2], csb["selqk3"][:, :], t62[:, :],
                             start=True, stop=True)
            nc.tensor.matmul(bca_ps[:, 2:4], csb["selv3"][:, :], t62[:, :],
                             start=True, stop=True)
            bca = pbc.tile([128, 4], F32, name="bca", tag="bca")
            nc.vector.tensor_copy(bca[:, :], bca_ps[:, :])
            S["bca"] = bca
            S["bcv"] = bca[:, 2:4]

            # per-sample row-scaled qlin/klin (rsQ rows 0:64, rsK rows 64:128)
            qklin_s = pqs.tile([128, N_PIX], BF16, name="qklin_s", tag="qs")
            nc.vector.tensor_scalar_mul(qklin_s[:, :], qklin_bf[:, :],
                                        bca[:, 1:2])
            S["qklin_s"] = qklin_s


        def front_c2(S):
            """Projections; -mu folded into the PSUM->SBUF add."""
            feats = S["feats"]
            bca = S["bca"]
            bcqk = bca[:, 0:2]
            bcv = bca[:, 2:4]
            stacked = []
            for h in range(HEADS):
                st_t = pstk.tile([128, N_PIX], BF16, name="st_t", tag="qk")
                stacked.append(st_t)
                pps = ps_fr.tile([128, 512], F32, name="pps", tag="fr")
                nc.tensor.matmul(pps[:, :], kqw2e_bf[:, h * 128:(h + 1) * 128],
                                 feats[:, 0:512], start=True, stop=True)
                pps2 = ps_e.tile([128, 84], F32, name="pps2", tag="ep")
                nc.tensor.matmul(pps2[:, :], kqw2e_bf[:, h * 128:(h + 1) * 128],
                                 feats[:, 512:596], start=True, stop=True)
                nc.vector.tensor_scalar_add(st_t[:, 0:512], pps[:, :],
                                            bcqk[:, 0:1])
                nc.vector.tensor_scalar_add(st_t[:, 512:596], pps2[:, :],
                                            bcqk[:, 0:1])

            # V = (vps - muV) * s[c2]: Act Identity with per-partition
            # scale s and bias -muV*s (prepped once per sample).
            msv = pst.tile([128, 5], F32, name="msv", tag="msv")
            nc.vector.tensor_scalar(msv[:, :], csb["s_cols"][:, :],
                                    bcv[:, 0:1], None, op0=ALU.mult)
            vtiles = []
            for ci, (c0, c1) in enumerate(CH):
                csz = c1 - c0
                vps = ps_fr.tile([128, 256], F32, name="vps", tag="fr")
                nc.tensor.matmul(vps[0:csz, :], feats[:, c0:c1],
                                 vwe_bf[:, :], start=True, stop=True)
                vt = pv.tile([128, 512], BF16, name="vt", tag="v")
                vt3 = vt.rearrange("p (h c) -> p h c", c=128)
                vps3 = vps.rearrange("p (h c) -> p h c", c=64)
                nc.scalar.activation(vt3[0:csz, :, 0:64], vps3[0:csz, :, :],
                                     AF.Identity,
                                     bias=msv[0:csz, ci:ci + 1],
                                     scale=csb["s_cols"][0:csz, ci:ci + 1])
                vtiles.append(vt)
            S["stacked"] = stacked
            S["vtiles"] = vtiles
            S["eall"] = [peall.tile([65, N_PIX], BF16, name=f"eall{i}",
                                    tag="eall") for i in range(HEADS)]
            stats = ptl.tile([128, 10], F32, name="stats128", tag="stats")
            nc.vector.memset(stats[64:128, :], 0.0)
            S["stats128"] = stats
            S["pm"] = ptl.tile([1, 5, 64], F32, name="pm", tag="pm")
            return S

        # ---- attention stages (pipeline carried across samples) ----
        def at_pair_mm(S, p, pi):
            atp = ps_atp.tile([128, 2, 512], F32, name="atp", tag="atp")
            for j in range(2):
                ci = 2 * pi + j
                c0, c1 = CH[ci]
                if p["merged"]:
                    for h in range(HEADS):
                        nc.tensor.matmul(atp[:, j, h * 84:(h + 1) * 84],
                                         S["qklin_s"][:, c0:c1],
                                         S["stacked"][h][:, 512:596],
                                         start=True, stop=True)
                else:
                    nc.tensor.matmul(atp[:, j, 0:512],
                                     S["qklin_s"][:, c0:c1],
                                     S["stacked"][p["h"]][:, 0:512],
                                     start=True, stop=True)
            return atp

        def at_pair_ew(p, pi, atp, dest_pair):
            """Paired exp -> per-half min-mult + combine."""
            w = 512 if not p["merged"] else W84
            et = pet.tile([128, 2, 512], BF16, name="et", tag="et")
            nc.scalar.activation(et[:, :, 0:w], atp[:, :, 0:w], AF.Exp)
            for j in range(2):
                ci = 2 * pi + j
                eng = nc.gpsimd if MIN_POOL[ci] else nc.vector
                eng.tensor_scalar(et[:, j, 0:w], et[:, j, 0:w],
                                  csb["ebtab"][:, ci:ci + 1], 1.0,
                                  op0=ALU.mult, op1=ALU.min)
                nc.vector.scalar_tensor_tensor(
                    dest_pair[:, j, 0:w], atp[:, j, 0:w],
                    csb["b1tab"][:, ci:ci + 1],
                    et[:, j, 0:w], op0=ALU.add, op1=ALU.max)

        def at_c4_mm(S, p):
            c0, c1 = CH[4]
            atc = ps_atp.tile([128, 512], F32, name="atc", tag="atp")
            if p["merged"]:
                for h in range(HEADS):
                    nc.tensor.matmul(atc[0:84, h * 84:(h + 1) * 84],
                                     S["qklin_s"][:, c0:c1],
                                     S["stacked"][h][:, 512:596],
                                     start=True, stop=True)
            else:
                nc.tensor.matmul(atc[0:84, 0:512], S["qklin_s"][:, c0:c1],
                                 S["stacked"][p["h"]][:, 0:512],
                                 start=True, stop=True)
            return atc

        def at_c4_ew(p, atc, dest):
            w = 512 if not p["merged"] else W84
            et = pet.tile([128, 512], BF16, name="et4", tag="et4")
            nc.scalar.activation(et[0:84, 0:w], atc[0:84, 0:w], AF.Exp)
            eng = nc.gpsimd if MIN_POOL[4] else nc.vector
            eng.tensor_scalar(et[0:84, 0:w], et[0:84, 0:w],
                              csb["ebtab"][0:84, 4:5], 1.0,
                              op0=ALU.mult, op1=ALU.min)
            nc.vector.scalar_tensor_tensor(
                dest[0:84, 0:w], atc[0:84, 0:w], csb["b1tab"][0:84, 4:5],
                et[0:84, 0:w], op0=ALU.add, op1=ALU.max)

        def e_c2(st, c2i):
            S, p, tiles = st["S"], st["p"], st["tiles"]
            c20, c21 = CH[c2i]
            c2sz = c21 - c20
            w = 512 if not p["merged"] else W84
            if c2i == 0:
                st["eps"] = ps_eps.tile([128, 512], F32, name="eps_t", tag="e")
            eps_t = st["eps"]
            a2ps = ps_e.tile([128, 512], F32, name="a2ps", tag="ep")
            for j in range(2):
                nc.tensor.matmul(
                    a2ps[0:128, 0:w],
                    alin_i8[j][:, 256 * c2i:256 * c2i + 256],
                    tiles[j][:, :, 0:w],
                    start=(j == 0), stop=False,
                    perf_mode=mybir.MatmulPerfMode.DoubleRowSwInterleave)
            nc.tensor.matmul(a2ps[0:128, 0:w],
                             alin_bf4[:, 128 * c2i:128 * c2i + 128],
                             tiles[2][0:84, 0:w],
                             start=False, stop=True)
            ext = pext.tile([128, 512], BF16, name="ext", tag="ext")
            nc.scalar.activation(ext[0:c2sz, 0:w], a2ps[0:c2sz, 0:w], AF.Exp,
                                 scale=1.0 / ALSC)
            if p["merged"]:
                # PSUM accumulation groups must not interleave within a
                # bank's zero region: buffer ext tiles, accumulate in e_tail.
                st.setdefault("exts", []).append(ext)
            else:
                nc.tensor.matmul(eps_t[:, 0:512],
                                 S["vtiles"][c2i][0:c2sz,
                                                  p["h"] * 128:
                                                  (p["h"] + 1) * 128],
                                 ext[0:c2sz, 0:512],
                                 start=(c2i == 0), stop=(c2i == 4))

        def e_tail(st):
            """Normalize each head's E by its own softmax denominator
            (times rsV); eall row 64 holds a persistent ones-row that
            carries the lin1 bias through the transposed lin1 stage."""
            S, p, eps_t = st["S"], st["p"], st["eps"]
            w = 512 if not p["merged"] else W84
            eall = S["eall"]
            rsv = S["bca"][0:64, 3:4]
            if p["merged"]:
                for h in range(HEADS):
                    for c2i, (c20, c21) in enumerate(CH):
                        c2sz = c21 - c20
                        nc.tensor.matmul(
                            eps_t[:, h * 84:(h + 1) * 84],
                            S["vtiles"][c2i][0:c2sz, h * 128:(h + 1) * 128],
                            st["exts"][c2i][0:c2sz, h * 84:(h + 1) * 84],
                            start=(c2i == 0), stop=(c2i == 4))
            recip64 = pst.tile([64, 512], F32, name="recip64", tag="recip")
            nc.vector.reciprocal(recip64[:, 0:w], eps_t[64:128, 0:w])
            if p["merged"]:
                for h in range(HEADS):
                    nc.vector.scalar_tensor_tensor(
                        eall[h][0:64, 512:596],
                        eps_t[0:64, h * 84:(h + 1) * 84], rsv,
                        recip64[:, h * 84:(h + 1) * 84],
                        op0=ALU.mult, op1=ALU.mult)
            else:
                h = p["h"]
                nc.vector.scalar_tensor_tensor(
                    eall[h][0:64, 0:512], eps_t[0:64, 0:512], rsv,
                    recip64[:, 0:512], op0=ALU.mult, op1=ALU.mult)

        pending = [None]

        def do_pass(S, p):
            pair0 = pat.tile([128, 2, 512], FP8, name="atp0", tag="atile")
            pair1 = pat.tile([128, 2, 512], FP8, name="atp1", tag="atile")
            at4 = pat.tile([128, 512], BF16, name="at4", tag="a4", bufs=2)
            tiles = [pair0, pair1, at4]
            prev = pending[0]
            # PE: this pass's at matmuls first; Act: prev pass's e-exps
            # flow while the at matmuls run (no head-of-line blocking).
            atp0 = at_pair_mm(S, p, 0)
            if prev is None:
                atp1 = at_pair_mm(S, p, 1)
                atc = at_c4_mm(S, p)
                at_pair_ew(p, 0, atp0, pair0)
                at_pair_ew(p, 1, atp1, pair1)
                at_c4_ew(p, atc, at4)
            else:
                e_c2(prev, 0)
                atp1 = at_pair_mm(S, p, 1)
                e_c2(prev, 1)
                at_pair_ew(p, 0, atp0, pair0)
                e_c2(prev, 2)
                atc = at_c4_mm(S, p)
                e_c2(prev, 3)
                at_pair_ew(p, 1, atp1, pair1)
                e_c2(prev, 4)
                at_c4_ew(p, atc, at4)
                e_tail(prev)
            pending[0] = {"S": S, "p": p, "tiles": tiles}

        def flush_pipe():
            prev = pending[0]
            for c2i in range(5):
                e_c2(prev, c2i)
            e_tail(prev)
            pending[0] = None

        def tail_blk(S, fb):
            """Transposed lin1 for f-block fb: out[f, do] with the softmax
            denominator riding row 64 of the eall copies; normalization via
            a per-partition reciprocal scale in the relu."""
            f0 = fb * 128
            fsz = min(128, N_PIX - f0)
            eall, stats = S["eall"], S["stats128"]
            lps2 = ps_e.tile([128, 64], F32, name="lps2", tag="ep")
            for h in range(HEADS):
                nc.tensor.matmul(lps2[0:fsz, :], eall[h][0:64, f0:f0 + fsz],
                                 lin1w_bf[:, 64 * h:64 * (h + 1)],
                                 start=(h == 0), stop=False)
            nc.tensor.matmul(lps2[0:fsz, :], eall[0][64:65, f0:f0 + fsz],
                             csb["bl1r65"][64:65, :], start=False, stop=True)
            e2t = psq.tile([128, 64], F32, name="e2t", tag="e2t")
            nc.scalar.activation(e2t[0:fsz, :], lps2[0:fsz, :], AF.Relu,
                                 accum_out=stats[0:fsz, fb:fb + 1])
            sq2 = psq.tile([128, 64], F32, name="sq2", tag="sq2")
            nc.gpsimd.tensor_tensor(sq2[0:fsz, :], e2t[0:fsz, :],
                                    e2t[0:fsz, :], op=ALU.mult)
            nc.vector.tensor_reduce(stats[0:fsz, 5 + fb:6 + fb],
                                    sq2[0:fsz, :],
                                    axis=mybir.AxisListType.X, op=ALU.add)
            nc.gpsimd.tensor_reduce(S["pm"][0:1, fb, :], e2t[0:fsz, :],
                                    axis=mybir.AxisListType.C, op=ALU.max)

        def tail_fin(S):
            """Combine per-block stats, LN2 scalars, normalized max-pool."""
            s, stats = S["s"], S["stats128"]
            st10_ps = ps_e.tile([1, 10], F32, name="st10_ps", tag="ep")
            nc.tensor.matmul(st10_ps[0:1, :], csb["ones_c"][:, 0:1],
                             stats[:, :], start=True, stop=True)
            ls2 = pst.tile([1, 2], F32, name="ls2", tag="ls2")
            nc.vector.tensor_reduce(
                ls2[:, :].rearrange("p (a u) -> p a u", u=1),
                st10_ps[0:1, :].rearrange("p (a b) -> p a b", a=2),
                axis=mybir.AxisListType.X, op=ALU.add)
            emax_do = pst.tile([1, 64], F32, name="emax_do", tag="emax_do")
            nc.vector.tensor_reduce(
                emax_do[0:1, :].rearrange("p (b u) -> p b u", u=1),
                S["pm"][0:1, :, :].rearrange("p a b -> p b a"),
                axis=mybir.AxisListType.X, op=ALU.max)
            emt_ps = ps_e.tile([64, 1], F32, name="emt_ps", tag="ep")
            nc.tensor.transpose(emt_ps[:, :], emax_do[0:1, :],
                                csb["ones_c"][0:1, 0:1])
            # per-sample LN2 scalars (mean/var -> Newton rsqrt)
            m2 = pst.tile([1, 2], F32, name="m2", tag="m2")
            nc.vector.tensor_scalar_mul(m2[:, :], ls2[:, :], 1.0 / LN2_N)
            ve = pst.tile([1, 2], F32, name="ve", tag="ve")
            nc.vector.scalar_tensor_tensor(ve[:, 1:2], m2[:, 0:1], -1.0,
                                           m2[:, 0:1],
                                           op0=ALU.mult, op1=ALU.mult)
            nc.vector.scalar_tensor_tensor(ve[:, 0:1], m2[:, 1:2], EPS,
                                           ve[:, 1:2], op0=ALU.add,
                                           op1=ALU.add)
            yk2 = pst.tile([1, 1], I32, name="yk2", tag="yk2")
            nc.vector.tensor_scalar(yk2[:, :], ve.bitcast(I32)[:, 0:1],
                                    1, None, op0=ALU.logical_shift_right)
            nc.vector.tensor_scalar(yk2[:, :], yk2[:, :], -1, RSQ_MAGIC,
                                    op0=ALU.mult, op1=ALU.add)
            y2 = yk2.bitcast(F32)
            nt2 = pst.tile([1, 1], F32, name="nt2", tag="nt2")
            for _ in range(2):
                nc.vector.tensor_tensor(nt2[:, :], y2[:, :], y2[:, :],
                                        op=ALU.mult)
                nc.vector.tensor_tensor(nt2[:, :], nt2[:, :], ve[:, 0:1],
                                        op=ALU.mult)
                nc.vector.tensor_scalar(nt2[:, :], nt2[:, :], -0.5, 1.5,
                                        op0=ALU.mult, op1=ALU.add)
                nc.vector.tensor_tensor(y2[:, :], y2[:, :], nt2[:, :],
                                        op=ALU.mult)
            rsnm = pst.tile([1, 2], F32, name="rsnm", tag="rsnm")
            nc.vector.tensor_copy(rsnm[:, 0:1], y2[:, :])
            nc.vector.scalar_tensor_tensor(rsnm[:, 1:2], m2[:, 0:1], -1.0,
                                           y2[:, :], op0=ALU.mult,
                                           op1=ALU.mult)
            bc2_ps = ps_e.tile([64, 2], F32, name="bc2_ps", tag="ep")
            nc.tensor.matmul(bc2_ps[:, :], csb["ones_r"][0:1, 0:64],
                             rsnm[:, :], start=True, stop=True)
            bc2s = pst.tile([64, 2], F32, name="bc2s", tag="bc2s")
            nc.vector.tensor_copy(bc2s[:, :], bc2_ps[:, :])
            emt = pst.tile([64, 1], F32, name="emt", tag="emt")
            nc.vector.tensor_copy(emt[:, :], emt_ps[:, :])
            nc.vector.tensor_scalar(emax_all[:, s:s + 1], emt[:, :],
                                    bc2s[:, 0:1], bc2s[:, 1:2],
                                    op0=ALU.mult, op1=ALU.add)

        # ---- pipelined schedule: sample s+1's front-end is emitted between
        # sample s's attention passes; the at/e pass pipeline is carried
        # across the sample boundary.
        S = front_a(0)
        front_b(S)
        front_c1(S)
        front_c2(S)
        states = {0: S}
        for s in range(spb):
            S = states[s]
            plist = ([dict(h=h, merged=False) for h in range(HEADS)]
                     + [dict(h=None, merged=True)])
            do_pass(S, plist[0])
            if s > 0:
                Sp = states.pop(s - 1)
                tail_blk(Sp, 4)
                tail_fin(Sp)
            if s + 1 < spb:
                Sn = front_a(s + 1)
            do_pass(S, plist[1])
            if s + 1 < spb:
                front_b(Sn)
            do_pass(S, plist[2])
            if s + 1 < spb:
                front_c1(Sn)
            do_pass(S, plist[3])
            if s + 1 < spb:
                front_c2(Sn)
                states[s + 1] = Sn
            do_pass(S, plist[4])
            for fb in range(4):
                tail_blk(S, fb)
        flush_pipe()
        Sp = states.pop(spb - 1)
        tail_blk(Sp, 4)
        tail_fin(Sp)

        # ---------------- lin2 + final elu ----------------
        l2ps = ps_e.tile([10, spb], F32, name="l2ps", tag="ep")
        nc.tensor.matmul(l2ps[:, :], csb["lin2w"][:, :], emax_all[:, :],
                         start=True, stop=True)
        fe = pst.tile([10, spb], F32, name="fe", tag="fe")
        nc.scalar.activation(fe[:, :], l2ps[:, :], AF.Exp,
                             bias=csb["bl2"][:, 0:1])
        nc.vector.tensor_scalar(fe[:, :], fe[:, :], 1.0, -1.0,
                                op0=ALU.min, op1=ALU.add)
        out_sb = pst.tile([10, spb], F32, name="out_sb", tag="out_sb")
        nc.vector.scalar_tensor_tensor(out_sb[:, :], l2ps[:, :],
                                       csb["bl2"][:, 0:1], fe[:, :],
                                       op0=ALU.add, op1=ALU.max)
        nc.sync.dma_start(out=out_dram.rearrange("s t -> t s"), in_=out_sb[:, :])

    return nc


def _reference_numpy(inp):
    """Pure-numpy fallback (only used if LN affine params are nontrivial)."""
    def ln(x, g=None, b=None):
        axes = tuple(range(1, x.ndim))
        mu = x.mean(axis=axes, keepdims=True)
        var = x.var(axis=axes, keepdims=True)
        y = (x - mu) / np.sqrt(var + EPS)
        return y * g + b if g is not None else y

    def elu(x):
        return np.where(x > 0, x, np.expm1(np.minimum(x, 0)))

    x = np.asarray(inp["x"], np.float64)
    N = x.shape[0]
    w1, b1 = np.asarray(inp["conv1_w"], np.float64), np.asarray(inp["conv1_b"], np.float64)
    h = np.zeros((N, 16, 150, 5))
    for di in range(2):
        for dj in range(2):
            h += np.einsum("oc,nchw->nohw", w1[:, :, di, dj],
                           x[:, :, di:di + 150, dj:dj + 5])
    h = np.maximum(h + b1[None, :, None, None], 0)
    w2, b2 = np.asarray(inp["conv2_w"], np.float64), np.asarray(inp["conv2_b"], np.float64)
    h2 = np.zeros((N, 32, 149, 4))
    for di in range(2):
        for dj in range(2):
            h2 += np.einsum("oc,nchw->nohw", w2[:, :, di, dj],
                            h[:, :, di:di + 149, dj:dj + 4])
    h2 = np.maximum(h2 + b2[None, :, None, None], 0)
    p = np.arange(N_PIX)
    xc, yc = (p % 4) / 4.0, (p // 4) / 149.0
    feats = np.concatenate(
        [h2.transpose(0, 2, 3, 1).reshape(N, N_PIX, 32),
         np.broadcast_to(np.stack([xc, yc], 1)[None], (N, N_PIX, 2))], axis=2)

    def proj(wn, bn, gn, bn2):
        P = (feats @ np.asarray(inp[wn], np.float64) + np.asarray(inp[bn], np.float64))
        P = P.reshape(N, N_PIX, HEADS, D).transpose(0, 2, 1, 3)
        return ln(P, np.asarray(inp[gn], np.float64), np.asarray(inp[bn2], np.float64))

    K = proj("kp_w", "kp_b", "knorm_g", "knorm_b")
    Q = proj("qp_w", "qp_b", "qnorm_g", "qnorm_b")
    V = proj("vp_w", "vp_b", "vnorm_g", "vnorm_b")
    A = elu(Q @ np.asarray(inp["qlin_w"], np.float64) + np.asarray(inp["qlin_b"], np.float64)
            + K @ np.asarray(inp["klin_w"], np.float64) + np.asarray(inp["klin_b"], np.float64))
    A = A @ np.asarray(inp["alin_w"], np.float64) + np.asarray(inp["alin_b"], np.float64)
    A = A - A.max(axis=-1, keepdims=True)
    A = np.exp(A)
    A = A / A.sum(axis=-1, keepdims=True)
    E = np.einsum("bhfc,bhcd->bhfd", A, V)
    E = E.transpose(0, 2, 1, 3).reshape(N, N_PIX, HEADS * D)
    E = np.maximum(E @ np.asarray(inp["lin1_w"], np.float64)
                   + np.asarray(inp["lin1_b"], np.float64), 0)
    E = ln(E)
    E = E.max(axis=1)
    out = E @ np.asarray(inp["lin2_w"], np.float64) + np.asarray(inp["lin2_b"], np.float64)
    return elu(out).astype(np.float32)


def kernel(**inputs):
    trivial = (np.all(np.asarray(inputs["knorm_g"]) == 1.0)
               and np.all(np.asarray(inputs["knorm_b"]) == 0.0)
               and np.all(np.asarray(inputs["qnorm_g"]) == 1.0)
               and np.all(np.asarray(inputs["qnorm_b"]) == 0.0)
               and np.all(np.asarray(inputs["vnorm_g"]) == 1.0)
               and np.all(np.asarray(inputs["vnorm_b"]) == 0.0))
    if not trivial:
        return _reference_numpy(inputs)

    x = np.ascontiguousarray(np.asarray(inputs["x"], np.float32))
    n = x.shape[0]
    assert n == N_CORES * SPB, f"expected batch {N_CORES * SPB}, got {n}"
    consts = _prep_consts(inputs)

    if "nc" not in _cache:
        nc = build_nc(SPB)
        nc.compile()
        _cache["nc"] = nc
    nc = _cache["nc"]

    in_maps = []
    for c in range(N_CORES):
        m = dict(consts)
        m["x"] = np.ascontiguousarray(x[c * SPB:(c + 1) * SPB])
        in_maps.append(m)

    import os
    trace = bool(int(os.environ.get("KERNEL_TRACE", "0")))
    res = run_bass_kernel_spmd(nc, in_maps, list(range(N_CORES)), trace=trace)
    kernel._last_results = res
    out = np.concatenate([np.asarray(r["out"]) for r in res.results], axis=0)
    return out.astype(np.float32)


kernel._last_results = None


# revision 69
# speedup vs baseline: 1.0977x; 1.0231x over previous
"""Fused Trainium2 kernel for nn_MultiHeadRelationalModule.

Data-parallel over 8 NeuronCores (8 samples each). The whole per-sample
pipeline (conv1 -> conv2 -> +coords -> K/Q/V proj -> LayerNorm ->
relational attention (4 heads, 596x596) -> softmax -> weighted sum ->
lin1 -> LN -> maxpool -> lin2 -> elu) runs on-chip; the big attention
maps never touch HBM.

v3 rework (vs v2 baseline, 391.6us -> 380.3us):
  * Act runs only Exp/Relu/Identity (one act-table load; was 19 = 24us).
    All rsqrt via fast-inverse-sqrt bitcast + 2 Newton steps on DVE.
  * at-stage exp is bias-free (exp(z+b) = exp(z)*exp(b); exp(b) folds
    into the min-op scalar) and reads paired 2-bank PSUM tiles.
  * softmax-exp bias exp(alin_b - colsum(alin_w)) folds multiplicatively
    into the V tiles and their denominator ones-columns.
  * Q/K/V biases ride a 35th ones-row of feats through the projections;
    LN stats via Gram-trace <G, W W^T> against host-precomputed mats;
    LN means fold into the PSUM->SBUF adds, scales into a per-sample
    qlin/klin row-scaled copy and the softmax-normalize multiply.
  * lin1 runs transposed (f on partitions) with the bias riding a
    persistent ones-row of the eall copies; LN2 stats per-sample with
    partition-axis max on gpsimd.
  * do_pass emits all at-matmuls before the e-stage so Act's FIFO never
    head-of-line blocks; elementwise chains ride a single DVE queue.
  * NOTE hardware constraints found: GPSIMD (Pool) cannot access PSUM;
    tensor_tensor_reduce does not execute on HW; ALU pow/divide do not
    lower. PSUM-readers must be Act (0.83ns/elem) or DVE (1.04ns/elem).

Key identities:
  elu(x)+1 == max(x + 1, min(exp(x), 1))          (exact)
  A' = elu(z)+1 fed to matmul with alin_w: subtract colsum(alin_w) in
       the softmax bias to undo the +1; that bias is then moved out of
       the exp into a multiplicative row-scale on V.
  max-pool commutes with the final LN (monotone affine map).
"""

import numpy as np
from contextlib import ExitStack

import concourse.bacc as bacc
import concourse.bass as bass
import concourse.mybir as mybir
import concourse.tile as tile
from concourse.bass_utils import run_bass_kernel_spmd

F32 = mybir.dt.float32
BF16 = mybir.dt.bfloat16
FP8 = mybir.dt.float8e4
I32 = mybir.dt.int32
RSQ_MAGIC = 0x5F3759DF
ALSC = 16.0  # alin pre-scale into fp8e4m3 normal range; undone in exp scale
AF = mybir.ActivationFunctionType
ALU = mybir.AluOpType

N_CORES = 8
SPB = 8               # samples per core
N_PIX = 596
HEADS = 4
D = 64
CH = [(0, 128), (128, 256), (256, 384), (384, 512), (512, 596)]
FH = [(0, 512), (512, 596)]
SHIFTS = [(0, 0), (0, 1), (1, 0), (1, 1)]
LN_N = float(HEADS * N_PIX * D)       # 152576
LN2_N = float(N_PIX * D)              # 38144
EPS = 1e-5
W84 = 84 * HEADS

_cache = {}


def _prep_consts(inp):
    """Host-side preprocessing of weights into kernel-friendly layouts."""
    f = np.float32
    c = {}
    conv1_w = np.asarray(inp["conv1_w"], f)
    c["w1s"] = np.ascontiguousarray(
        np.concatenate([conv1_w[:, :, di, dj].T for (di, dj) in SHIFTS], axis=1)
    )  # (4, 64)
    c["b1"] = np.ascontiguousarray(np.asarray(inp["conv1_b"], f)[:, None])  # (16,1)
    conv2_w = np.asarray(inp["conv2_w"], f)
    c["w2s"] = np.ascontiguousarray(
        np.concatenate([conv2_w[:, :, di, dj].T for (di, dj) in SHIFTS], axis=1)
    )  # (16, 128)
    c["b2"] = np.ascontiguousarray(np.asarray(inp["conv2_b"], f)[:, None])  # (32,1)

    p = np.arange(N_PIX)
    c["coords3"] = np.ascontiguousarray(
        np.stack([(p % 4) / 4.0, (p // 4) / 149.0,
                  np.ones(N_PIX)]).astype(f)
    )  # (3, 596): xc, yc, ones-row (projection bias carrier)

    # Q/K projection merged per head with bias in a 35th feats-ones row:
    # cols h*128:h*128+64 = Q, cols h*128+64:h*128+128 = K.
    qp_w = np.asarray(inp["qp_w"], f)
    kp_w = np.asarray(inp["kp_w"], f)
    qp_b = np.asarray(inp["qp_b"], f)
    kp_b = np.asarray(inp["kp_b"], f)
    kqw2e = np.zeros((35, 512), f)
    for h in range(HEADS):
        kqw2e[0:34, h * 128:h * 128 + 64] = qp_w[:, h * 64:(h + 1) * 64]
        kqw2e[0:34, h * 128 + 64:h * 128 + 128] = kp_w[:, h * 64:(h + 1) * 64]
        kqw2e[34, h * 128:h * 128 + 64] = qp_b[h * 64:(h + 1) * 64]
        kqw2e[34, h * 128 + 64:h * 128 + 128] = kp_b[h * 64:(h + 1) * 64]
    c["kqw2e"] = kqw2e

    vwe = np.zeros((35, 256), f)
    vwe[0:34] = np.asarray(inp["vp_w"], f)
    vwe[34] = np.asarray(inp["vp_b"], f)
    c["vwe"] = vwe

    c["qklin"] = np.ascontiguousarray(
        np.concatenate([np.asarray(inp["qlin_w"], f),
                        np.asarray(inp["klin_w"], f)], axis=0)
    )  # (128, 596): rows 0:64 qlin (Q), 64:128 klin (K)

    qkl_b = np.asarray(inp["qlin_b"], f) + np.asarray(inp["klin_b"], f)
    b1tab = np.zeros((128, 5), f)
    ebtab = np.zeros((128, 5), f)
    for ci, (c0, c1) in enumerate(CH):
        b1tab[0:c1 - c0, ci] = qkl_b[c0:c1] + 1.0
        ebtab[0:c1 - c0, ci] = np.exp(qkl_b[c0:c1].astype(np.float64)).astype(f)
    c["b1tab"] = b1tab
    c["ebtab"] = ebtab

    # fp8e4m3 DoubleRowSwInterleave weight pairs for alin rows 0:512 (x16 so
    # the ~0.05-scale entries sit in e4m3's normal range; undone in exp scale).
    import ml_dtypes
    alin_w = np.asarray(inp["alin_w"], f)
    alin16 = np.pad(alin_w * ALSC, ((0, 0), (0, 44)))
    for j in range(2):
        A = alin16[256 * j:256 * j + 128]
        B = alin16[256 * j + 128:256 * j + 256]
        buf = np.zeros((128, 1280), f)
        for ci in range(5):
            c0 = 128 * ci
            blk = np.empty((128, 256), f)
            blk[:, 0::2] = A[:, c0:c0 + 128][:, ::-1]
            blk[:, 1::2] = B[:, c0:c0 + 128][:, ::-1]
            buf[:, 2 * c0:2 * c0 + 256] = blk
        c[f"alin_i8_{j}"] = np.ascontiguousarray(
            buf.astype(ml_dtypes.float8_e4m3))
    c["alin4"] = np.ascontiguousarray(
        np.pad(alin_w[512:596, :] * ALSC, ((0, 0), (0, 44))
               ).astype(ml_dtypes.bfloat16))  # (84, 640), pre-scaled

    # softmax bias exp(alin_b - colsum(alin_w)) folded into V rows (c2 dim)
    s = np.exp((np.asarray(inp["alin_b"], np.float64)
                - np.asarray(inp["alin_w"], np.float64).sum(axis=0)))
    s = s.astype(f)
    s_cols = np.zeros((128, 5), f)
    for ci, (c0, c1) in enumerate(CH):
        s_cols[0:c1 - c0, ci] = s[c0:c1]
    c["s_cols"] = s_cols

    lin1_w = np.asarray(inp["lin1_w"], f)
    l1 = np.zeros((64, 256), f)
    for h in range(HEADS):
        l1[:, h * 64:(h + 1) * 64] = lin1_w[h * 64:(h + 1) * 64, :]
    c["lin1w"] = l1
    c["bl1"] = np.ascontiguousarray(np.asarray(inp["lin1_b"], f)[:, None])  # (64,1)
    b65 = np.zeros((65, 64), f)
    b65[64, :] = np.asarray(inp["lin1_b"], f)
    c["bl1r65"] = np.ascontiguousarray(b65.astype(ml_dtypes.bfloat16))
    c["lin2w"] = np.ascontiguousarray(np.asarray(inp["lin2_w"], f))  # (64,10)
    bl2 = np.zeros((10, 2), f)
    bl2[:, 0] = np.asarray(inp["lin2_b"], f)
    bl2[:, 1] = np.asarray(inp["lin2_b"], f) + 1.0
    c["bl2"] = bl2
    c["ones_r"] = np.ones((1, 128), f)
    c["ones_c"] = np.ones((128, 1), f)
    c["epsc"] = np.full((1, 1), EPS, f)
    c["id35"] = np.eye(35, dtype=f)
    # row-group selectors: cols of mm(sel, t62) pick (-mu, rs) per partition
    selqk3 = np.zeros((3, 128), f)
    selqk3[0, 0:64] = 1.0
    selqk3[1, 64:128] = 1.0
    selv3 = np.zeros((3, 128), f)
    selv3[2, :] = 1.0
    c["selqk3"] = selqk3
    c["selv3"] = selv3
    # LN-stat helpers: per tensor T with extended weights W' (35, .):
    # sum(T) = s'^T W' 1 and ssq(T) = <G, W' W'^T> with G = feats' feats'^T
    # (biases included via the ones-row of feats').
    wq = np.concatenate([qp_w, qp_b[None]], 0)
    wk = np.concatenate([kp_w, kp_b[None]], 0)
    wsum3 = np.zeros((35, 3), f)
    wsum3[:, 0] = wq.sum(axis=1)
    wsum3[:, 1] = wk.sum(axis=1)
    wsum3[:, 2] = vwe.sum(axis=1)
    c["wsum3"] = wsum3.astype(ml_dtypes.bfloat16)
    for k in ("w1s", "w2s", "coords3", "kqw2e", "vwe", "qklin", "lin1w",
              "id35"):
        c[k] = c[k].astype(ml_dtypes.bfloat16)
    c["gmq"] = np.ascontiguousarray((wq @ wq.T).astype(f))   # (35, 35)
    c["gmk"] = np.ascontiguousarray((wk @ wk.T).astype(f))
    c["gmv"] = np.ascontiguousarray((vwe @ vwe.T).astype(f))
    return c


CONST_SHAPES = {
    "b1": (16, 1), "b2": (32, 1),
    "b1tab": (128, 5), "ebtab": (128, 5), "s_cols": (128, 5),
    "bl1": (64, 1), "lin2w": (64, 10),
    "bl2": (10, 2), "ones_r": (1, 128), "ones_c": (128, 1), "epsc": (1, 1),
    "selqk3": (3, 128), "selv3": (3, 128),
    "gmq": (35, 35), "gmk": (35, 35), "gmv": (35, 35),
}
CONST_BF16 = {
    "bl1r65": (65, 64),
    "w1s": (4, 64), "w2s": (16, 128), "coords3": (3, N_PIX),
    "kqw2e": (35, 512), "vwe": (35, 256), "qklin": (128, N_PIX),
    "lin1w": (64, 256), "alin4": (84, 640), "wsum3": (35, 3),
    "id35": (35, 35),
}
CONST_FP8 = {"alin_i8_0": (128, 1280), "alin_i8_1": (128, 1280)}

# elu min-op engine split per chunk: True -> Pool, False -> DVE (4x)
MIN_POOL = [False, False, False, False, False]


def build_nc(spb=SPB):
    """Build the Bass program (same program runs SPMD on each core)."""
    nc = bacc.Bacc("TRN2", target_bir_lowering=False, debug=False)

    x_dram = nc.dram_tensor("x", [spb, 4, 151, 6], F32, kind="ExternalInput").ap()
    out_dram = nc.dram_tensor("out", [spb, 10], F32, kind="ExternalOutput").ap()
    cdram = {
        k: nc.dram_tensor(k, list(v), F32, kind="ExternalInput").ap()
        for k, v in CONST_SHAPES.items()
    }
    for k, v in CONST_BF16.items():
        cdram[k] = nc.dram_tensor(k, list(v), BF16, kind="ExternalInput").ap()
    for k, v in CONST_FP8.items():
        cdram[k] = nc.dram_tensor(k, list(v), FP8, kind="ExternalInput").ap()

    with tile.TileContext(nc) as tc, ExitStack() as ctx:
        pc = ctx.enter_context(tc.tile_pool(name="consts", bufs=1))
        # SBUF pools
        px = ctx.enter_context(tc.tile_pool(name="px", bufs=2))
        ph1 = ctx.enter_context(tc.tile_pool(name="ph1", bufs=2))
        pfeat = ctx.enter_context(tc.tile_pool(name="pfeat", bufs=2))
        pstk = ctx.enter_context(tc.tile_pool(name="pstk", bufs=8))
        pqs = ctx.enter_context(tc.tile_pool(name="pqs", bufs=2))
        pbc = ctx.enter_context(tc.tile_pool(name="pbc", bufs=2))
        pv = ctx.enter_context(tc.tile_pool(name="pv", bufs=10))
        pet = ctx.enter_context(tc.tile_pool(name="pet", bufs=3))
        pat = ctx.enter_context(tc.tile_pool(name="pat", bufs=4))
        pext = ctx.enter_context(tc.tile_pool(name="pext", bufs=7))
        psq = ctx.enter_context(tc.tile_pool(name="psq", bufs=2))
        pst = ctx.enter_context(tc.tile_pool(name="pst", bufs=3))
        peall = ctx.enter_context(tc.tile_pool(name="peall", bufs=8))
        ptl = ctx.enter_context(tc.tile_pool(name="ptl", bufs=2))
        pfix = ctx.enter_context(tc.tile_pool(name="pfix", bufs=1))
        # PSUM pools: 8 banks = at-pairs 2x2 + e-ring 2 + eps 1 + fr 1.
        PS = bass.MemorySpace.PSUM
        ps_atp = ctx.enter_context(tc.tile_pool(name="ps_atp", bufs=2, space=PS))
        ps_e = ctx.enter_context(tc.tile_pool(name="ps_e", bufs=2, space=PS))
        ps_eps = ctx.enter_context(tc.tile_pool(name="ps_eps", bufs=1, space=PS))
        ps_fr = ctx.enter_context(tc.tile_pool(name="ps_fr", bufs=1, space=PS))

        # ---- prefetch sample 0's input before the const DMAs ----
        x_t0 = px.tile([4, 151, 6], F32, name="x_t", tag="x")
        nc.sync.dma_start(out=x_t0[:, :, :], in_=x_dram[0])

        # ---- load constants; critical-path consts first ----
        csb = {}
        first = ["w1s", "b1", "w2s", "b2", "coords3", "kqw2e", "vwe",
                 "qklin", "wsum3", "id35", "selqk3", "selv3", "s_cols",
                 "b1tab", "ebtab", "gmq", "gmk", "gmv"]
        order = first + [k for k in list(CONST_SHAPES) + list(CONST_BF16)
                         if k not in first]
        dmaq = [nc.sync, nc.scalar, nc.gpsimd]
        for i, k in enumerate(order):
            if k in CONST_SHAPES:
                shp, dt = CONST_SHAPES[k], F32
            else:
                shp, dt = CONST_BF16[k], BF16
            t = pc.tile(list(shp), dt, name=f"c_{k}")
            dmaq[i % 3].dma_start(out=t[:, :], in_=cdram[k][:, :])
            csb[k] = t
        alin_i8 = []
        for j in range(2):
            t = pc.tile([128, 1280], FP8, name=f"alin_i8_{j}")
            dmaq[j].dma_start(out=t[:, :], in_=cdram[f"alin_i8_{j}"][:, :])
            alin_i8.append(t)

        w1s_bf = csb["w1s"]
        w2s_bf = csb["w2s"]
        kqw2e_bf = csb["kqw2e"]
        vwe_bf = csb["vwe"]
        qklin_bf = csb["qklin"]
        lin1w_bf = csb["lin1w"]
        id35_bf = csb["id35"]
        alin_bf4 = csb["alin4"]
        wsum3_bf = csb["wsum3"]
        ones_bf = pc.tile([128, 1], BF16, name="ones_bf")
        nc.vector.memset(ones_bf[:, :], 1.0)
        # sB: per-c2-chunk softmax scale broadcast, built from s_cols
        ones256 = pc.tile([128, 256], BF16, name="ones256")
        nc.vector.memset(ones256[:, :], 1.0)
        sB_bf = pc.tile([128, 5, 256], BF16, name="sB_bf")
        for ci in range(5):
            nc.vector.tensor_scalar_mul(sB_bf[:, ci, :], ones256[:, :],
                                        csb["s_cols"][:, ci:ci + 1])
        sB3 = sB_bf
        # feats'-transpose staging tiles; col 35 is a persistent ones column
        # so the Gram matmul also yields the feature sums s'.
        ft_bufs = []
        for i in range(3):
            fb = pst.tile([128, 36], BF16, name="ft_sb", tag="ft")
            nc.vector.memset(fb[:, 35:36], 1.0)
            ft_bufs.append(fb)
        emax_all = pfix.tile([64, spb], F32, name="emax_all")

        # eall ring: row 64 is a persistent ones-row (bias carrier for the
        # transposed lin1); the per-pass normalize writes only rows 0:64.
        for i in range(8):
            eb_t = peall.tile([65, N_PIX], BF16, name="eall_i", tag="eall")
            nc.vector.memset(eb_t[64:65, :], 1.0)

        # feats tiles: conv writes rows 0:32; rows 32:34 coords, row 34 ones,
        # both persistent (written once into each ring buffer).
        feats_bufs = []
        for i in range(2):
            ft = pfeat.tile([35, N_PIX], BF16, name="feats", tag="feats")
            nc.vector.tensor_copy(ft[32:35, :], csb["coords3"][:, :])
            feats_bufs.append(ft)

        # V tiles: cols h*128+64:h*128+128 hold the persistent softmax-scale
        # block s[c2] (denominator ones-columns, pre-scaled).
        for i in range(10):
            vt = pv.tile([128, 512], BF16, name="vt", tag="v")
            vt3 = vt.rearrange("p (h c) -> p h c", c=128)
            ci = i % 5
            csz = CH[ci][1] - CH[ci][0]
            nc.vector.tensor_copy(
                vt3[0:csz, :, 64:128],
                sB3[0:csz, ci, :].rearrange("p (h d) -> p h d", h=4))

        # ================= pipelined per-sample stages =================

        def front_a(s):
            """x load/cast + conv1 + conv2 -> feats (relu on Pool)."""
            S = {"s": s}
            if s == 0:
                x_t = x_t0
            else:
                x_t = px.tile([4, 151, 6], F32, name="x_t", tag="x")
                nc.sync.dma_start(out=x_t[:, :, :], in_=x_dram[s])
            x_bf = px.tile([4, 151, 6], BF16, name="x_bf", tag="xbf")
            nc.gpsimd.tensor_copy(x_bf[:, :, :], x_t[:, :, :])

            h1 = ph1.tile([16, 750], BF16, name="h1", tag="h1")
            h1v = h1.rearrange("c (h w) -> c h w", w=5)
            for (r0, nr, dst0) in ((0, 102, 0), (102, 48, 510)):
                cps = ps_fr.tile([16, nr * 5], F32, name="c1ps", tag="fr")
                for si, (di, dj) in enumerate(SHIFTS):
                    nc.tensor.matmul(
                        cps[:, :],
                        w1s_bf[:, si * 16:(si + 1) * 16],
                        x_bf[:, di + r0:di + r0 + nr, dj:dj + 5],
                        start=(si == 0), stop=(si == 3),
                    )
                nc.scalar.activation(h1[:, dst0:dst0 + nr * 5], cps[:, :],
                                     AF.Relu, bias=csb["b1"][:, 0:1])

            feats = feats_bufs[s % 2]
            for (r0, nr, dst0) in ((0, 128, 0), (128, 21, 512)):
                cps = ps_fr.tile([32, nr * 4], F32, name="c2ps", tag="fr")
                for si, (di, dj) in enumerate(SHIFTS):
                    nc.tensor.matmul(
                        cps[:, :],
                        w2s_bf[:, si * 32:(si + 1) * 32],
                        h1v[:, di + r0:di + r0 + nr, dj:dj + 4],
                        start=(si == 0), stop=(si == 3),
                    )
                nc.scalar.activation(feats[0:32, dst0:dst0 + nr * 4],
                                     cps[:, :], AF.Relu,
                                     bias=csb["b2"][:, 0:1])
            S["feats"] = feats
            return S

        def front_b(S):
            """LN stats: G36 = [feats'|1]^T-gram on the PE (last col = s'),
            then ssq = <G, W W^T> via ttr against host Gram mats."""
            feats = S["feats"]
            g_ps = ps_fr.tile([36, 36], F32, name="g_ps", tag="fr")
            for ci, (c0, c1) in enumerate(CH):
                csz = c1 - c0
                ft_ps = ps_e.tile([128, 35], BF16, name="ft_ps", tag="ep")
                nc.tensor.transpose(ft_ps[0:csz, :], feats[:, c0:c1],
                                    id35_bf[:, :])
                ft_sb = ft_bufs[ci % 3]
                nc.vector.tensor_copy(ft_sb[0:csz, 0:35], ft_ps[0:csz, :])
                nc.tensor.matmul(g_ps[:, :], ft_sb[0:csz, :],
                                 ft_sb[0:csz, :],
                                 start=(ci == 0), stop=(ci == 4))
            g_sb = pst.tile([36, 36], BF16, name="g_sb", tag="g_sb")
            nc.vector.tensor_copy(g_sb[:, :], g_ps[:, :])
            gw = psq.tile([35, 3, 35], F32, name="gw", tag="gw")
            acc3 = pst.tile([35, 3], F32, name="acc3", tag="acc3")
            for i, gm in enumerate(("gmq", "gmk", "gmv")):
                nc.gpsimd.tensor_tensor(gw[:, i, :], g_sb[0:35, 0:35],
                                        csb[gm][:, :], op=ALU.mult)
            nc.vector.tensor_reduce(
                acc3[:, :].rearrange("p (a u) -> p a u", u=1),
                gw[:, :, :], axis=mybir.AxisListType.X, op=ALU.add)
            stats_ps = ps_fr.tile([1, 6], F32, name="stats_ps", tag="fr")
            nc.tensor.matmul(stats_ps[0:1, 0:3], g_sb[0:35, 35:36],
                             wsum3_bf[:, :], start=True, stop=True)
            nc.tensor.matmul(stats_ps[0:1, 3:6], csb["ones_c"][0:35, 0:1],
                             acc3[:, :], start=True, stop=True)
            mu3 = pst.tile([1, 3], F32, name="mu3", tag="mu3")
            nc.vector.tensor_scalar_mul(mu3[:, :], stats_ps[0:1, 0:3],
                                        1.0 / LN_N)
            msq3 = pst.tile([1, 3], F32, name="msq3", tag="msq3")
            nc.vector.tensor_scalar_mul(msq3[:, :], stats_ps[0:1, 3:6],
                                        1.0 / LN_N)
            S["mu3"] = mu3
            S["msq3"] = msq3
            return S

        def front_c1(S):
            """LN scalars via Newton rsqrt on DVE."""
            mu3, msq3 = S["mu3"], S["msq3"]
            nmu2 = pst.tile([1, 3], F32, name="nmu2", tag="nmu2")
            nc.vector.scalar_tensor_tensor(nmu2[:, :], mu3[:, :], -1.0,
                                           mu3[:, :],
                                           op0=ALU.mult, op1=ALU.mult)
            var3e = pst.tile([1, 3], F32, name="var3e", tag="var3e")
            nc.vector.scalar_tensor_tensor(var3e[:, :], msq3[:, :], EPS,
                                           nmu2[:, :], op0=ALU.add,
                                           op1=ALU.add)
            nm3 = pst.tile([1, 3], F32, name="nm3", tag="nm3")
            nc.vector.tensor_scalar_mul(nm3[:, :], mu3[:, :], -1.0)
            # transpose (-mu | var) onto partitions 0:3
            t6_ps = ps_fr.tile([3, 2], F32, name="t6_ps", tag="fr")
            nc.tensor.transpose(t6_ps[:, 0:1], nm3[0:1, :],
                                csb["ones_c"][0:1, 0:1])
            nc.tensor.transpose(t6_ps[:, 1:2], var3e[0:1, :],
                                csb["ones_c"][0:1, 0:1])
            t62 = pst.tile([3, 2], F32, name="t62", tag="t62")
            nc.vector.tensor_copy(t62[:, :], t6_ps[:, :])
            # fast inverse sqrt + 2 Newton steps: rs = (var+eps)^-0.5
            yk = pst.tile([3, 1], I32, name="yk", tag="yk")
            nc.vector.tensor_scalar(yk[:, :], t62.bitcast(I32)[:, 1:2],
                                    1, None, op0=ALU.logical_shift_right)
            nc.vector.tensor_scalar(yk[:, :], yk[:, :], -1, RSQ_MAGIC,
                                    op0=ALU.mult, op1=ALU.add)
            y = yk.bitcast(F32)
            nt = pst.tile([3, 1], F32, name="nt", tag="nt")
            for _ in range(2):
                nc.vector.tensor_tensor(nt[:, :], y[:, :], y[:, :],
                                        op=ALU.mult)
                nc.vector.tensor_tensor(nt[:, :], nt[:, :], t62[:, 1:2],
                                        op=ALU.mult)
                nc.vector.tensor_scalar(nt[:, :], nt[:, :], -0.5, 1.5,
                                        op0=ALU.mult, op1=ALU.add)
                nc.vector.tensor_tensor(y[:, :], y[:, :], nt[:, :],
                                        op=ALU.mult)
            nc.vector.tensor_copy(t62[:, 1:2], y[:, :])
            # broadcast (-mu, rs) to per-partition columns via row selectors
            bca_ps = ps_fr.tile([128, 4], F32, name="bca_ps", tag="fr")
            nc.tensor.matmul(bca_ps[:, 0:2], csb["selqk3"][:, :], t62[:, :],
                             start=True, stop=True)
            nc.tensor.matmul(bca_ps[:, 2:4], csb["selv3"][:, :], t62[:, :],
                             start=True, stop=True)
            bca = pbc.tile([128, 4], F32, name="bca", tag="bca")
            nc.vector.tensor_copy(bca[:, :], bca_ps[:, :])
            S["bca"] = bca
            S["bcv"] = bca[:, 2:4]

            # per-sample row-scaled qlin/klin (rsQ rows 0:64, rsK rows 64:128)
            qklin_s = pqs.tile([128, N_PIX], BF16, name="qklin_s", tag="qs")
            nc.vector.tensor_scalar_mul(qklin_s[:, :], qklin_bf[:, :],
                                        bca[:, 1:2])
            S["qklin_s"] = qklin_s


        def front_c2(S):
            """Projections; -mu folded into the PSUM->SBUF add."""
            feats = S["feats"]
            bca = S["bca"]
            bcqk = bca[:, 0:2]
            bcv = bca[:, 2:4]
            stacked = []
            for h in range(HEADS):
                st_t = pstk.tile([128, N_PIX], BF16, name="st_t", tag="qk")
                stacked.append(st_t)
                pps = ps_fr.tile([128, 512], F32, name="pps", tag="fr")
                nc.tensor.matmul(pps[:, :], kqw2e_bf[:, h * 128:(h + 1) * 128],
                                 feats[:, 0:512], start=True, stop=True)
                pps2 = ps_e.tile([128, 84], F32, name="pps2", tag="ep")
                nc.tensor.matmul(pps2[:, :], kqw2e_bf[:, h * 128:(h + 1) * 128],
                                 feats[:, 512:596], start=True, stop=True)
                nc.vector.tensor_scalar_add(st_t[:, 0:512], pps[:, :],
                                            bcqk[:, 0:1])
                nc.scalar.activation(st_t[:, 512:596], pps2[:, :],
                                     AF.Identity, bias=bcqk[:, 0:1])

            # V = (vps - muV) * s[c2]: Act Identity with per-partition
            # scale s and bias -muV*s (prepped once per sample).
            msv = pst.tile([128, 5], F32, name="msv", tag="msv")
            nc.vector.tensor_scalar(msv[:, :], csb["s_cols"][:, :],
                                    bcv[:, 0:1], None, op0=ALU.mult)
            vtiles = []
            for ci, (c0, c1) in enumerate(CH):
                csz = c1 - c0
                vps = ps_fr.tile([128, 256], F32, name="vps", tag="fr")
                nc.tensor.matmul(vps[0:csz, :], feats[:, c0:c1],
                                 vwe_bf[:, :], start=True, stop=True)
                vt = pv.tile([128, 512], BF16, name="vt", tag="v")
                vt3 = vt.rearrange("p (h c) -> p h c", c=128)
                vps3 = vps.rearrange("p (h c) -> p h c", c=64)
                nc.scalar.activation(vt3[0:csz, :, 0:64], vps3[0:csz, :, :],
                                     AF.Identity,
                                     bias=msv[0:csz, ci:ci + 1],
                                     scale=csb["s_cols"][0:csz, ci:ci + 1])
                vtiles.append(vt)
            S["stacked"] = stacked
            S["vtiles"] = vtiles
            S["eall"] = [peall.tile([65, N_PIX], BF16, name=f"eall{i}",
                                    tag="eall") for i in range(HEADS)]
            stats = ptl.tile([128, 10], F32, name="stats128", tag="stats")
            nc.vector.memset(stats[64:128, :], 0.0)
            S["stats128"] = stats
            S["pm"] = ptl.tile([1, 5, 64], F32, name="pm", tag="pm")
            return S

        # ---- attention stages (pipeline carried across samples) ----
        def at_pair_mm(S, p, pi):
            atp = ps_atp.tile([128, 2, 512], F32, name="atp", tag="atp")
            for j in range(2):
                ci = 2 * pi + j
                c0, c1 = CH[ci]
                if p["merged"]:
                    for h in range(HEADS):
                        nc.tensor.matmul(atp[:, j, h * 84:(h + 1) * 84],
                                         S["qklin_s"][:, c0:c1],
                                         S["stacked"][h][:, 512:596],
                                         start=True, stop=True)
                else:
                    nc.tensor.matmul(atp[:, j, 0:512],
                                     S["qklin_s"][:, c0:c1],
                                     S["stacked"][p["h"]][:, 0:512],
                                     start=True, stop=True)
            return atp

        def at_pair_ew(p, pi, atp, dest_pair):
            """Paired exp -> per-half min-mult + combine."""
            w = 512 if not p["merged"] else W84
            et = pet.tile([128, 2, 512], BF16, name="et", tag="et")
            nc.scalar.activation(et[:, :, 0:w], atp[:, :, 0:w], AF.Exp)
            for j in range(2):
                ci = 2 * pi + j
                eng = nc.gpsimd if MIN_POOL[ci] else nc.vector
                eng.tensor_scalar(et[:, j, 0:w], et[:, j, 0:w],
                                  csb["ebtab"][:, ci:ci + 1], 1.0,
                                  op0=ALU.mult, op1=ALU.min)
                nc.vector.scalar_tensor_tensor(
                    dest_pair[:, j, 0:w], atp[:, j, 0:w],
                    csb["b1tab"][:, ci:ci + 1],
                    et[:, j, 0:w], op0=ALU.add, op1=ALU.max)

        def at_c4_mm(S, p):
            c0, c1 = CH[4]
            atc = ps_atp.tile([128, 512], F32, name="atc", tag="atp")
            if p["merged"]:
                for h in range(HEADS):
                    nc.tensor.matmul(atc[0:84, h * 84:(h + 1) * 84],
                                     S["qklin_s"][:, c0:c1],
                                     S["stacked"][h][:, 512:596],
                                     start=True, stop=True)
            else:
                nc.tensor.matmul(atc[0:84, 0:512], S["qklin_s"][:, c0:c1],
                                 S["stacked"][p["h"]][:, 0:512],
                                 start=True, stop=True)
            return atc

        def at_c4_ew(p, atc, dest):
            w = 512 if not p["merged"] else W84
            et = pet.tile([128, 512], BF16, name="et4", tag="et4")
            nc.scalar.activation(et[0:84, 0:w], atc[0:84, 0:w], AF.Exp)
            eng = nc.gpsimd if MIN_POOL[4] else nc.vector
            eng.tensor_scalar(et[0:84, 0:w], et[0:84, 0:w],
                              csb["ebtab"][0:84, 4:5], 1.0,
                              op0=ALU.mult, op1=ALU.min)
            nc.vector.scalar_tensor_tensor(
                dest[0:84, 0:w], atc[0:84, 0:w], csb["b1tab"][0:84, 4:5],
                et[0:84, 0:w], op0=ALU.add, op1=ALU.max)

        def e_c2(st, c2i):
            S, p, tiles = st["S"], st["p"], st["tiles"]
            c20, c21 = CH[c2i]
            c2sz = c21 - c20
            w = 512 if not p["merged"] else W84
            if c2i == 0:
                st["eps"] = ps_eps.tile([128, 512], F32, name="eps_t", tag="e")
            eps_t = st["eps"]
            a2ps = ps_e.tile([128, 512], F32, name="a2ps", tag="ep")
            for j in range(2):
                nc.tensor.matmul(
                    a2ps[0:128, 0:w],
                    alin_i8[j][:, 256 * c2i:256 * c2i + 256],
                    tiles[j][:, :, 0:w],
                    start=(j == 0), stop=False,
                    perf_mode=mybir.MatmulPerfMode.DoubleRowSwInterleave)
            nc.tensor.matmul(a2ps[0:128, 0:w],
                             alin_bf4[:, 128 * c2i:128 * c2i + 128],
                             tiles[2][0:84, 0:w],
                             start=False, stop=True)
            ext = pext.tile([128, 512], BF16, name="ext", tag="ext")
            nc.scalar.activation(ext[0:c2sz, 0:w], a2ps[0:c2sz, 0:w], AF.Exp,
                                 scale=1.0 / ALSC)
            if p["merged"]:
                # PSUM accumulation groups must not interleave within a
                # bank's zero region: buffer ext tiles, accumulate in e_tail.
                st.setdefault("exts", []).append(ext)
            else:
                nc.tensor.matmul(eps_t[:, 0:512],
                                 S["vtiles"][c2i][0:c2sz,
                                                  p["h"] * 128:
                                                  (p["h"] + 1) * 128],
                                 ext[0:c2sz, 0:512],
                                 start=(c2i == 0), stop=(c2i == 4))

        def e_tail(st):
            """Normalize each head's E by its own softmax denominator
            (times rsV); eall row 64 holds a persistent ones-row that
            carries the lin1 bias through the transposed lin1 stage."""
            S, p, eps_t = st["S"], st["p"], st["eps"]
            w = 512 if not p["merged"] else W84
            eall = S["eall"]
            rsv = S["bca"][0:64, 3:4]
            if p["merged"]:
                for h in range(HEADS):
                    for c2i, (c20, c21) in enumerate(CH):
                        c2sz = c21 - c20
                        nc.tensor.matmul(
                            eps_t[:, h * 84:(h + 1) * 84],
                            S["vtiles"][c2i][0:c2sz, h * 128:(h + 1) * 128],
                            st["exts"][c2i][0:c2sz, h * 84:(h + 1) * 84],
                            start=(c2i == 0), stop=(c2i == 4))
            recip64 = pst.tile([64, 512], F32, name="recip64", tag="recip")
            nc.vector.reciprocal(recip64[:, 0:w], eps_t[64:128, 0:w])
            if p["merged"]:
                for h in range(HEADS):
                    nc.vector.scalar_tensor_tensor(
                        eall[h][0:64, 512:596],
                        eps_t[0:64, h * 84:(h + 1) * 84], rsv,
                        recip64[:, h * 84:(h + 1) * 84],
                        op0=ALU.mult, op1=ALU.mult)
            else:
                h = p["h"]
                nc.vector.scalar_tensor_tensor(
                    eall[h][0:64, 0:512], eps_t[0:64, 0:512], rsv,
                    recip64[:, 0:512], op0=ALU.mult, op1=ALU.mult)

        pending = [None]

        def do_pass(S, p):
            pair0 = pat.tile([128, 2, 512], FP8, name="atp0", tag="atile")
            pair1 = pat.tile([128, 2, 512], FP8, name="atp1", tag="atile")
            at4 = pat.tile([128, 512], BF16, name="at4", tag="a4", bufs=2)
            tiles = [pair0, pair1, at4]
            prev = pending[0]
            # PE: this pass's at matmuls first; Act: prev pass's e-exps
            # flow while the at matmuls run (no head-of-line blocking).
            atp0 = at_pair_mm(S, p, 0)
            if prev is None:
                atp1 = at_pair_mm(S, p, 1)
                atc = at_c4_mm(S, p)
                at_pair_ew(p, 0, atp0, pair0)
                at_pair_ew(p, 1, atp1, pair1)
                at_c4_ew(p, atc, at4)
            else:
                e_c2(prev, 0)
                atp1 = at_pair_mm(S, p, 1)
                at_pair_ew(p, 0, atp0, pair0)
                e_c2(prev, 1)
                e_c2(prev, 2)
                atc = at_c4_mm(S, p)
                at_c4_ew(p, atc, at4)
                e_c2(prev, 3)
                at_pair_ew(p, 1, atp1, pair1)
                e_c2(prev, 4)
                e_tail(prev)
            pending[0] = {"S": S, "p": p, "tiles": tiles}

        def flush_pipe():
            prev = pending[0]
            for c2i in range(5):
                e_c2(prev, c2i)
            e_tail(prev)
            pending[0] = None

        def tail_blk(S, fb):
            """Transposed lin1 for f-block fb: out[f, do] with the softmax
            denominator riding row 64 of the eall copies; normalization via
            a per-partition reciprocal scale in the relu."""
            f0 = fb * 128
            fsz = min(128, N_PIX - f0)
            eall, stats = S["eall"], S["stats128"]
            lps2 = ps_e.tile([128, 64], F32, name="lps2", tag="ep")
            for h in range(HEADS):
                nc.tensor.matmul(lps2[0:fsz, :], eall[h][0:64, f0:f0 + fsz],
                                 lin1w_bf[:, 64 * h:64 * (h + 1)],
                                 start=(h == 0), stop=False)
            nc.tensor.matmul(lps2[0:fsz, :], eall[0][64:65, f0:f0 + fsz],
                             csb["bl1r65"][64:65, :], start=False, stop=True)
            e2t = psq.tile([128, 64], F32, name="e2t", tag="e2t")
            nc.scalar.activation(e2t[0:fsz, :], lps2[0:fsz, :], AF.Relu,
                                 accum_out=stats[0:fsz, fb:fb + 1])
            sq2 = psq.tile([128, 64], F32, name="sq2", tag="sq2")
            nc.gpsimd.tensor_tensor(sq2[0:fsz, :], e2t[0:fsz, :],
                                    e2t[0:fsz, :], op=ALU.mult)
            nc.vector.tensor_reduce(stats[0:fsz, 5 + fb:6 + fb],
                                    sq2[0:fsz, :],
                                    axis=mybir.AxisListType.X, op=ALU.add)
            nc.gpsimd.tensor_reduce(S["pm"][0:1, fb, :], e2t[0:fsz, :],
                                    axis=mybir.AxisListType.C, op=ALU.max)

        def tail_fin(S):
            """Combine per-block stats, LN2 scalars, normalized max-pool."""
            s, stats = S["s"], S["stats128"]
            st10_ps = ps_e.tile([1, 10], F32, name="st10_ps", tag="ep")
            nc.tensor.matmul(st10_ps[0:1, :], csb["ones_c"][:, 0:1],
                             stats[:, :], start=True, stop=True)
            ls2 = pst.tile([1, 2], F32, name="ls2", tag="ls2")
            nc.vector.tensor_reduce(
                ls2[:, :].rearrange("p (a u) -> p a u", u=1),
                st10_ps[0:1, :].rearrange("p (a b) -> p a b", a=2),
                axis=mybir.AxisListType.X, op=ALU.add)
            emax_do = pst.tile([1, 64], F32, name="emax_do", tag="emax_do")
            nc.vector.tensor_reduce(
                emax_do[0:1, :].rearrange("p (b u) -> p b u", u=1),
                S["pm"][0:1, :, :].rearrange("p a b -> p b a"),
                axis=mybir.AxisListType.X, op=ALU.max)
            emt_ps = ps_e.tile([64, 1], F32, name="emt_ps", tag="ep")
            nc.tensor.transpose(emt_ps[:, :], emax_do[0:1, :],
                                csb["ones_c"][0:1, 0:1])
            # per-sample LN2 scalars (mean/var -> Newton rsqrt)
            m2 = pst.tile([1, 2], F32, name="m2", tag="m2")
            nc.vector.tensor_scalar_mul(m2[:, :], ls2[:, :], 1.0 / LN2_N)
            ve = pst.tile([1, 2], F32, name="ve", tag="ve")
            nc.vector.scalar_tensor_tensor(ve[:, 1:2], m2[:, 0:1], -1.0,
                                           m2[:, 0:1],
                                           op0=ALU.mult, op1=ALU.mult)
            nc.vector.scalar_tensor_tensor(ve[:, 0:1], m2[:, 1:2], EPS,
                                           ve[:, 1:2], op0=ALU.add,
                                           op1=ALU.add)
            yk2 = pst.tile([1, 1], I32, name="yk2", tag="yk2")
            nc.vector.tensor_scalar(yk2[:, :], ve.bitcast(I32)[:, 0:1],
                                    1, None, op0=ALU.logical_shift_right)
            nc.vector.tensor_scalar(yk2[:, :], yk2[:, :], -1, RSQ_MAGIC,
                                    op0=ALU.mult, op1=ALU.add)
            y2 = yk2.bitcast(F32)
            nt2 = pst.tile([1, 1], F32, name="nt2", tag="nt2")
            for _ in range(2):
                nc.vector.tensor_tensor(nt2[:, :], y2[:, :], y2[:, :],
                                        op=ALU.mult)
                nc.vector.tensor_tensor(nt2[:, :], nt2[:, :], ve[:, 0:1],
                                        op=ALU.mult)
                nc.vector.tensor_scalar(nt2[:, :], nt2[:, :], -0.5, 1.5,
                                        op0=ALU.mult, op1=ALU.add)
                nc.vector.tensor_tensor(y2[:, :], y2[:, :], nt2[:, :],
                                        op=ALU.mult)
            rsnm = pst.tile([1, 2], F32, name="rsnm", tag="rsnm")
            nc.vector.tensor_copy(rsnm[:, 0:1], y2[:, :])
            nc.vector.scalar_tensor_tensor(rsnm[:, 1:2], m2[:, 0:1], -1.0,
                                           y2[:, :], op0=ALU.mult,
                                           op1=ALU.mult)
            bc2_ps = ps_e.tile([64, 2], F32, name="bc2_ps", tag="ep")
            nc.tensor.matmul(bc2_ps[:, :], csb["ones_r"][0:1, 0:64],
                             rsnm[:, :], start=True, stop=True)
            bc2s = pst.tile([64, 2], F32, name="bc2s", tag="bc2s")
            nc.vector.tensor_copy(bc2s[:, :], bc2_ps[:, :])
            emt = pst.tile([64, 1], F32, name="emt", tag="emt")
            nc.vector.tensor_copy(emt[:, :], emt_ps[:, :])
            nc.vector.tensor_scalar(emax_all[:, s:s + 1], emt[:, :],
                                    bc2s[:, 0:1], bc2s[:, 1:2],
                                    op0=ALU.mult, op1=ALU.add)

        # ---- pipelined schedule: sample s+1's front-end is emitted between
        # sample s's attention passes; the at/e pass pipeline is carried
        # across the sample boundary.
        S = front_a(0)
        front_b(S)
        front_c1(S)
        front_c2(S)
        states = {0: S}
        for s in range(spb):
            S = states[s]
            plist = ([dict(h=h, merged=False) for h in range(HEADS)]
                     + [dict(h=None, merged=True)])
            do_pass(S, plist[0])
            if s > 0:
                Sp = states.pop(s - 1)
                tail_blk(Sp, 4)
                tail_fin(Sp)
            if s + 1 < spb:
                Sn = front_a(s + 1)
            do_pass(S, plist[1])
            if s + 1 < spb:
                front_b(Sn)
            do_pass(S, plist[2])
            if s + 1 < spb:
                front_c1(Sn)
            do_pass(S, plist[3])
            if s + 1 < spb:
                front_c2(Sn)
                states[s + 1] = Sn
            do_pass(S, plist[4])
            for fb in range(4):
                tail_blk(S, fb)
        flush_pipe()
        Sp = states.pop(spb - 1)
        tail_blk(Sp, 4)
        tail_fin(Sp)

        # ---------------- lin2 + final elu ----------------
        l2ps = ps_e.tile([10, spb], F32, name="l2ps", tag="ep")
        nc.tensor.matmul(l2ps[:, :], csb["lin2w"][:, :], emax_all[:, :],
                         start=True, stop=True)
        fe = pst.tile([10, spb], F32, name="fe", tag="fe")
        nc.scalar.activation(fe[:, :], l2ps[:, :], AF.Exp,
                             bias=csb["bl2"][:, 0:1])
        nc.vector.tensor_scalar(fe[:, :], fe[:, :], 1.0, -1.0,
                                op0=ALU.min, op1=ALU.add)
        out_sb = pst.tile([10, spb], F32, name="out_sb", tag="out_sb")
        nc.vector.scalar_tensor_tensor(out_sb[:, :], l2ps[:, :],
                                       csb["bl2"][:, 0:1], fe[:, :],
                                       op0=ALU.add, op1=ALU.max)
        nc.sync.dma_start(out=out_dram.rearrange("s t -> t s"), in_=out_sb[:, :])

    return nc


def _reference_numpy(inp):
    """Pure-numpy fallback (only used if LN affine params are nontrivial)."""
    def ln(x, g=None, b=None):
        axes = tuple(range(1, x.ndim))
        mu = x.mean(axis=axes, keepdims=True)
        var = x.var(axis=axes, keepdims=True)
        y = (x - mu) / np.sqrt(var + EPS)
        return y * g + b if g is not None else y

    def elu(x):
        return np.where(x > 0, x, np.expm1(np.minimum(x, 0)))

    x = np.asarray(inp["x"], np.float64)
    N = x.shape[0]
    w1, b1 = np.asarray(inp["conv1_w"], np.float64), np.asarray(inp["conv1_b"], np.float64)
    h = np.zeros((N, 16, 150, 5))
    for di in range(2):
        for dj in range(2):
            h += np.einsum("oc,nchw->nohw", w1[:, :, di, dj],
                           x[:, :, di:di + 150, dj:dj + 5])
    h = np.maximum(h + b1[None, :, None, None], 0)
    w2, b2 = np.asarray(inp["conv2_w"], np.float64), np.asarray(inp["conv2_b"], np.float64)
    h2 = np.zeros((N, 32, 149, 4))
    for di in range(2):
        for dj in range(2):
            h2 += np.einsum("oc,nchw->nohw", w2[:, :, di, dj],
                            h[:, :, di:di + 149, dj:dj + 4])
    h2 = np.maximum(h2 + b2[None, :, None, None], 0)
    p = np.arange(N_PIX)
    xc, yc = (p % 4) / 4.0, (p // 4) / 149.0
    feats = np.concatenate(
        [h2.transpose(0, 2, 3, 1).reshape(N, N_PIX, 32),
         np.broadcast_to(np.stack([xc, yc], 1)[None], (N, N_PIX, 2))], axis=2)

    def proj(wn, bn, gn, bn2):
        P = (feats @ np.asarray(inp[wn], np.float64) + np.asarray(inp[bn], np.float64))
        P = P.reshape(N, N_PIX, HEADS, D).transpose(0, 2, 1, 3)
        return ln(P, np.asarray(inp[gn], np.float64), np.asarray(inp[bn2], np.float64))

    K = proj("kp_w", "kp_b", "knorm_g", "knorm_b")
    Q = proj("qp_w", "qp_b", "qnorm_g", "qnorm_b")
    V = proj("vp_w", "vp_b", "vnorm_g", "vnorm_b")
    A = elu(Q @ np.asarray(inp["qlin_w"], np.float64) + np.asarray(inp["qlin_b"], np.float64)
            + K @ np.asarray(inp["klin_w"], np.float64) + np.asarray(inp["klin_b"], np.float64))
    A = A @ np.asarray(inp["alin_w"], np.float64) + np.asarray(inp["alin_b"], np.float64)
    A = A - A.max(axis=-1, keepdims=True)
    A = np.exp(A)
    A = A / A.sum(axis=-1, keepdims=True)
    E = np.einsum("bhfc,bhcd->bhfd", A, V)
    E = E.transpose(0, 2, 1, 3).reshape(N, N_PIX, HEADS * D)
    E = np.maximum(E @ np.asarray(inp["lin1_w"], np.float64)
                   + np.asarray(inp["lin1_b"], np.float64), 0)
    E = ln(E)
    E = E.max(axis=1)
    out = E @ np.asarray(inp["lin2_w"], np.float64) + np.asarray(inp["lin2_b"], np.float64)
    return elu(out).astype(np.float32)


def kernel(**inputs):
    trivial = (np.all(np.asarray(inputs["knorm_g"]) == 1.0)
               and np.all(np.asarray(inputs["knorm_b"]) == 0.0)
               and np.all(np.asarray(inputs["qnorm_g"]) == 1.0)
               and np.all(np.asarray(inputs["qnorm_b"]) == 0.0)
               and np.all(np.asarray(inputs["vnorm_g"]) == 1.0)
               and np.all(np.asarray(inputs["vnorm_b"]) == 0.0))
    if not trivial:
        return _reference_numpy(inputs)

    x = np.ascontiguousarray(np.asarray(inputs["x"], np.float32))
    n = x.shape[0]
    assert n == N_CORES * SPB, f"expected batch {N_CORES * SPB}, got {n}"
    consts = _prep_consts(inputs)

    if "nc" not in _cache:
        nc = build_nc(SPB)
        nc.compile()
        _cache["nc"] = nc
    nc = _cache["nc"]

    in_maps = []
    for c in range(N_CORES):
        m = dict(consts)
        m["x"] = np.ascontiguousarray(x[c * SPB:(c + 1) * SPB])
        in_maps.append(m)

    import os
    trace = bool(int(os.environ.get("KERNEL_TRACE", "0")))
    res = run_bass_kernel_spmd(nc, in_maps, list(range(N_CORES)), trace=trace)
    kernel._last_results = res
    out = np.concatenate([np.asarray(r["out"]) for r in res.results], axis=0)
    return out.astype(np.float32)


kernel._last_results = None
